# revision 46
# baseline (speedup 1.0000x reference)
"""Trainium2 Bass kernel for a 16-expert top-2 MoE layer with shared expert.

Sharding (8 cores):
  - Routed experts: expert-parallel, 2 experts per core, host-side token
    dispatch (gather) with a static per-expert capacity of C tokens.
  - Shared expert: tensor-parallel over the 4096-wide intermediate dim
    (512 per core); partial outputs summed on host.
  - Router runs on host (tiny: [2048,1024]@[1024,16]).

All matmul operands run in bf16 (fp32 PSUM accumulation); end-to-end
relative error ~4e-3.  bf16 halves every weight stream vs the fp32r
baseline and enables fast-weight-load on all stationaries, which takes
DMA off the critical path: the kernel is PE-issue-bound at ~1 moving
column/cycle.

Program order interleaves shared-expert chunks between routed-expert
steps so the PE never idles while weights stream in, and weight streams
for the next phase are prefetched during the previous one (wA1 during
S2, w2(e0) during late S1) to avoid phase-boundary stalls.
Streamed routed weights go on the sync HWDGE queue; activations,
resident shared weights and the e0 w2 stream on the scalar HWDGE queue;
outputs on the gpsimd SWDGE queue.
"""
import sys
sys.path.insert(0, "/opt/trn_rl_repo")
import numpy as np
import ml_dtypes
from concourse import bacc, mybir
from concourse import tile
from concourse import bass_utils

# Problem shape (hardcoded per contract)
B, T, D = 2, 1024, 1024
N = B * T                # 2048 tokens
E = 16                   # routed experts
H = 2048                 # expert intermediate
K = 2                    # top-k
HS = 4096                # shared intermediate
NCORES = 8
EPC = E // NCORES        # experts per core = 2
HSS = HS // NCORES       # shared intermediate slice per core = 512

P = 128
C = 256                  # per-expert token capacity (mean load is 256)
CT = (C + P - 1) // P    # 2 token tiles per expert
DT = D // P              # 8 contraction slices over D
HT = H // P              # 16 H tiles
HST = HSS // P           # 4 shared-H tiles per core
SCH = 512                # shared-expert token chunk (moving dim)
NCH = N // SCH           # 4 chunks
DN = D // 512            # 2 output free tiles of 512

F32 = mybir.dt.float32
BF16 = mybir.dt.bfloat16
AF = mybir.ActivationFunctionType

_CACHED = {}


def _mtiles():
    """Token tiles of (offset, width) covering C in 128-row steps."""
    out = []
    off = 0
    while off < C:
        out.append((off, min(P, C - off)))
        off += P
    return out


def _build_nc():
    nc = bacc.Bacc("TRN2", target_bir_lowering=False, debug=False,
                   num_devices=NCORES)

    # ---- DRAM I/O (per-core) ----
    xg = nc.dram_tensor("xg", [EPC, P, DT * C], BF16, kind="ExternalInput").ap()
    gt = nc.dram_tensor("gt", [P, EPC * CT], F32, kind="ExternalInput").ap()
    wa = nc.dram_tensor("wa", [EPC, HT, P, 2 * DT * P], BF16,
                        kind="ExternalInput").ap()
    w2 = nc.dram_tensor("w2", [EPC, HT, P, D], BF16, kind="ExternalInput").ap()
    b1 = nc.dram_tensor("b1", [P, EPC * HT], F32, kind="ExternalInput").ap()
    b11 = nc.dram_tensor("b11", [P, EPC * HT], F32, kind="ExternalInput").ap()
    xs = nc.dram_tensor("xs", [P, DT, N], BF16, kind="ExternalInput").ap()
    wsa = nc.dram_tensor("wsa", [HST, P, 2 * DT * P], BF16,
                         kind="ExternalInput").ap()
    ws2 = nc.dram_tensor("ws2", [HST, P, D], BF16, kind="ExternalInput").ap()
    bsa = nc.dram_tensor("bsa", [P, 2 * HST], F32, kind="ExternalInput").ap()
    rout = nc.dram_tensor("rout", [EPC, CT, P, D], BF16,
                          kind="ExternalOutput").ap()
    zout = nc.dram_tensor("zout", [N // P, P, D], BF16,
                          kind="ExternalOutput").ap()

    mtiles = _mtiles()

    with tile.TileContext(nc) as tc:
        with tc.tile_pool(name="sb", bufs=1) as sb, \
             tc.tile_pool(name="ps", bufs=1, space="PSUM") as ps:

            xg_t = [None, None]

            def load_xg(j, eng, split=False):
                t = sb.tile([P, DT * C], BF16, tag=f"xg{j}", name=f"xg_t{j}")
                if split:
                    eng.dma_start(t[:, :DT * C // 2], xg[j, :, 0:DT * C // 2])
                    eng.dma_start(t[:, DT * C // 2:], xg[j, :, DT * C // 2:])
                else:
                    eng.dma_start(t[:], xg[j])
                xg_t[j] = t

            wa_t = {}

            def load_wa(j, hi, eng, split=False):
                t = sb.tile([P, 2 * DT * P], BF16, tag="wa", bufs=8,
                            name=f"wa_t{j}_{hi}")
                if split:
                    eng.dma_start(t[:, :DT * P], wa[j, hi, :, 0:DT * P])
                    eng.dma_start(t[:, DT * P:], wa[j, hi, :, DT * P:])
                else:
                    eng.dma_start(t[:], wa[j, hi])
                wa_t[(j, hi)] = t

            w2_t = {}

            def load_w2(j, hi, eng):
                t = sb.tile([P, D], BF16, tag="w2", bufs=8,
                            name=f"w2_t{j}_{hi}")
                eng.dma_start(t[:], w2[j, hi])
                w2_t[(j, hi)] = t

            wsa_t = [None] * HST
            ws2_t = [None] * HST

            def load_wsa(hi, eng):
                t = sb.tile([P, 2 * DT * P], BF16, tag=f"wsa_{hi}",
                            name=f"wsa_t{hi}")
                eng.dma_start(t[:], wsa[hi])
                wsa_t[hi] = t

            def load_ws2(hi, eng):
                t = sb.tile([P, D], BF16, tag=f"ws2_{hi}", name=f"ws2_t{hi}")
                eng.dma_start(t[:], ws2[hi])
                ws2_t[hi] = t

            xs_tiles = {}

            def load_xs(t, eng, split=1):
                xs_t = sb.tile([P, DT * SCH], BF16, tag="xs", bufs=2,
                               name=f"xs_t{t}")
                qd = DT // split
                for q in range(split):
                    eng.dma_start(
                        xs_t[:, q * qd * SCH:(q + 1) * qd * SCH].rearrange(
                            "p (ds s) -> p ds s", ds=qd),
                        xs[:, q * qd:(q + 1) * qd, t * SCH:(t + 1) * SCH])
                xs_tiles[t] = xs_t

            h_t = {0: [], 1: []}
            s_t = {}

            def gen_routed_a(j, dma_filler=None):
                """16 steps, one H-tile each: stream wa, 2 psum groups,
                silu/bias-add/mul -> hT[hi] (bf16)."""
                for hi in range(HT):
                    if (j, hi) not in wa_t:
                        load_wa(j, hi, nc.sync)
                    if dma_filler is not None:
                        dma_filler(hi)
                    wt = wa_t[(j, hi)]
                    ps1 = ps.tile([P, 512], F32, tag="p1", bufs=2,
                                  name=f"ps1_{j}_{hi}")
                    ps2 = ps.tile([P, 512], F32, tag="p2", bufs=2,
                                  name=f"ps2_{j}_{hi}")
                    if j == 0 and hi == 0:
                        # first step: emit in DMA piece-arrival order
                        # (w1 half, xg half 1, w11 half, xg half 2)
                        halves = [(ps1, 0, range(0, DT // 2)),
                                  (ps2, DT * P, range(0, DT // 2)),
                                  (ps1, 0, range(DT // 2, DT)),
                                  (ps2, DT * P, range(DT // 2, DT))]
                        for pst, woff, dss in halves:
                            for ds in dss:
                                nc.tensor.matmul(
                                    pst[:, :C],
                                    wt[:, woff + ds * P:woff + (ds + 1) * P],
                                    xg_t[j][:, ds * C:(ds + 1) * C],
                                    start=(ds == 0), stop=(ds == DT - 1))
                    else:
                        for ds in range(DT):
                            nc.tensor.matmul(
                                ps1[:, :C],
                                wt[:, ds * P:(ds + 1) * P],
                                xg_t[j][:, ds * C:(ds + 1) * C],
                                start=(ds == 0), stop=(ds == DT - 1))
                        for ds in range(DT):
                            nc.tensor.matmul(
                                ps2[:, :C],
                                wt[:, DT * P + ds * P:DT * P + (ds + 1) * P],
                                xg_t[j][:, ds * C:(ds + 1) * C],
                                start=(ds == 0), stop=(ds == DT - 1))
                    t1 = sb.tile([P, C], F32, tag="t1r", bufs=2,
                                 name=f"t1_{j}_{hi}")
                    nc.scalar.activation(t1[:], ps1[:, :C], AF.Silu,
                                         bias=b1_all[:, j * HT + hi:j * HT + hi + 1])
                    t2 = sb.tile([P, C], F32, tag="t2r", bufs=2,
                                 name=f"t2_{j}_{hi}")
                    nc.vector.tensor_scalar_add(
                        t2[:], ps2[:, :C],
                        b11_all[:, j * HT + hi:j * HT + hi + 1])
                    ht = sb.tile([P, C], BF16, tag=f"h_{hi}", name=f"h_{j}_{hi}")
                    nc.vector.tensor_mul(ht[:], t1[:], t2[:])
                    h_t[j].append(ht)
                    yield

            def gen_routed_b(j, dma_filler=None, w2_eng=None):
                """16 steps (one per streamed w2 tile) + epilogue step."""
                pos = [[ps.tile([P, 512], F32, tag="po", bufs=4,
                                name=f"po_{j}_{m}_{n2}")
                        for n2 in range(DN)] for m in range(len(mtiles))]
                for hi in range(HT):
                    if (j, hi) not in w2_t:
                        load_w2(j, hi, w2_eng or nc.sync)
                    if dma_filler is not None:
                        dma_filler(hi)
                    w2t = w2_t[(j, hi)]
                    for m, (off, mw) in enumerate(mtiles):
                        for n2 in range(DN):
                            nc.tensor.matmul(
                                pos[m][n2][:mw, :],
                                h_t[j][hi][:, off:off + mw],
                                w2t[:, n2 * 512:(n2 + 1) * 512],
                                start=(hi == 0), stop=(hi == HT - 1))
                    yield
                spread = j == EPC - 1   # final expert: minimize kernel tail
                for i, (m, n2) in enumerate(
                        (m, n2) for m in range(len(mtiles)) for n2 in range(DN)):
                    off, mw = mtiles[m]
                    o_t = sb.tile([P, 512], BF16, tag="ot", bufs=4,
                                  name=f"o_t{j}_{m}_{n2}")
                    gsc = g_all[:mw, j * CT + m:j * CT + m + 1]
                    if spread and i >= 2:
                        nc.vector.tensor_scalar_mul(
                            o_t[:mw, :], pos[m][n2][:mw, :], gsc)
                    else:
                        nc.scalar.activation(
                            o_t[:mw, :], pos[m][n2][:mw, :], AF.Identity,
                            scale=gsc)
                    oe = nc.gpsimd
                    if spread:
                        oe = nc.sync if i < 2 else nc.gpsimd
                    oe.dma_start(
                        rout[j, m, 0:mw, n2 * 512:(n2 + 1) * 512],
                        o_t[:mw, :])
                yield

            def gen_shared_sa(t, p_tags, dma_filler=None):
                """4 steps, one shared-H tile each."""
                s_t[t] = []
                for hi in range(HST):
                    if dma_filler is not None:
                        dma_filler(hi)
                    ps1 = ps.tile([P, 512], F32, tag=p_tags[0],
                                  bufs=4 if p_tags[0] == "po" else 2,
                                  name=f"sps1_{t}_{hi}")
                    ps2 = ps.tile([P, 512], F32, tag=p_tags[1],
                                  bufs=4 if p_tags[1] == "po" else 2,
                                  name=f"sps2_{t}_{hi}")
                    for ds in range(DT):
                        nc.tensor.matmul(ps1[:],
                                         wsa_t[hi][:, ds * P:(ds + 1) * P],
                                         xs_tiles[t][:, ds * SCH:(ds + 1) * SCH],
                                         start=(ds == 0), stop=(ds == DT - 1))
                    for ds in range(DT):
                        nc.tensor.matmul(
                            ps2[:],
                            wsa_t[hi][:, DT * P + ds * P:DT * P + (ds + 1) * P],
                            xs_tiles[t][:, ds * SCH:(ds + 1) * SCH],
                            start=(ds == 0), stop=(ds == DT - 1))
                    t1 = sb.tile([P, 512], F32, tag="t1", bufs=2,
                                 name=f"st1_{t}_{hi}")
                    nc.scalar.activation(t1[:], ps1[:], AF.Silu,
                                         bias=bs_all[:, hi:hi + 1])
                    t2 = sb.tile([P, 512], F32, tag="t2", bufs=2,
                                 name=f"st2_{t}_{hi}")
                    nc.vector.tensor_scalar_add(t2[:], ps2[:],
                                                bs_all[:, HST + hi:HST + hi + 1])
                    st = sb.tile([P, 512], BF16, tag=f"s_{hi}", bufs=2,
                                 name=f"s_{t}_{hi}")
                    nc.vector.tensor_mul(st[:], t1[:], t2[:])
                    s_t[t].append(st)
                    yield

            def gen_shared_sb(t, p_tags, fine=False):
                """(token-tile, D-half) groups; 2 per step, or 1 if fine."""
                groups = [(mm, n2) for mm in range(SCH // P) for n2 in range(DN)]
                step = 1 if fine else 2
                for k in range(0, len(groups), step):
                    for gi, (mm, n2) in enumerate(groups[k:k + step], k):
                        _tg = p_tags[gi // 2 % 2]
                        po = ps.tile([P, 512], F32, tag=_tg,
                                     bufs=4 if _tg == "po" else 2,
                                     name=f"spo_{t}_{mm}_{n2}")
                        for hi in range(HST):
                            nc.tensor.matmul(
                                po[:],
                                s_t[t][hi][:, mm * P:(mm + 1) * P],
                                ws2_t[hi][:, n2 * 512:(n2 + 1) * 512],
                                start=(hi == 0), stop=(hi == HST - 1))
                        z_t = sb.tile([P, 512], BF16, tag="zt", bufs=4,
                                      name=f"z_t{t}_{mm}_{n2}")
                        if gi % 2:
                            nc.vector.tensor_copy(z_t[:], po[:])
                        else:
                            nc.scalar.activation(z_t[:], po[:], AF.Identity)
                        z_eng = nc.sync if fine and gi == 6 else nc.gpsimd
                        z_eng.dma_start(
                            zout[t * (SCH // P) + mm, :, n2 * 512:(n2 + 1) * 512],
                            z_t[:])
                    yield

            def drive(main_gen, fill_gen, sched):
                """Consume steps per explicit schedule string, then drain both."""
                for ch in sched:
                    g = main_gen if ch == "M" else fill_gen
                    try:
                        next(g)
                    except StopIteration:
                        pass
                for g in (main_gen, fill_gen):
                    while True:
                        try:
                            next(g)
                        except StopIteration:
                            break

            def chain(*gens):
                for g in gens:
                    yield from g

            # --- prologue: critical-path loads first, split fine and
            # interleaved so the first matmuls start as early as possible ---
            # sync: first wa tile + xg0 (feeds rA0 step 0), pieces ordered
            # by when rA0 step 0's matmuls consume them.
            wa00 = sb.tile([P, 2 * DT * P], BF16, tag="wa", bufs=8,
                           name="wa_t0_0")
            xg0t = sb.tile([P, DT * C], BF16, tag="xg0", name="xg_t0")
            nc.sync.dma_start(wa00[:, :DT * P], wa[0, 0, :, 0:DT * P])
            nc.sync.dma_start(xg0t[:, :DT * C // 2], xg[0, :, 0:DT * C // 2])
            nc.sync.dma_start(wa00[:, DT * P:], wa[0, 0, :, DT * P:])
            nc.sync.dma_start(xg0t[:, DT * C // 2:], xg[0, :, DT * C // 2:])
            wa_t[(0, 0)] = wa00
            xg_t[0] = xg0t
            # scalar: biases (step-0 epilogue), then SA0 inputs (~4us in)
            b1_all = sb.tile([P, EPC * HT], F32, name="b1_all")
            nc.scalar.dma_start(b1_all[:], b1[:])
            b11_all = sb.tile([P, EPC * HT], F32, name="b11_all")
            nc.scalar.dma_start(b11_all[:], b11[:])
            bs_all = sb.tile([P, 2 * HST], F32, name="bs_all")
            nc.scalar.dma_start(bs_all[:], bsa[:])
            load_wsa(0, nc.scalar)
            load_xs(0, nc.scalar, split=2)
            load_wsa(1, nc.scalar)
            load_wsa(2, nc.scalar)
            load_wsa(3, nc.scalar)
            g_all = sb.tile([P, EPC * CT], F32, name="g_all")
            nc.scalar.dma_start(g_all[:], gt[:])

            # S1: rA0 (streams wa on sync) interleaved with SA0+SA1.
            # scalar fillers: rest of shared residents, xs1, xg1, then
            # prefetch of w2(e0) for S2.
            def filler_a0(hi):
                if hi == 0:
                    load_xs(1, nc.scalar)
                elif hi == 1:
                    load_ws2(0, nc.sync)
                    load_ws2(1, nc.sync)
                elif hi == 2:
                    load_ws2(2, nc.sync)
                    load_ws2(3, nc.sync)
                elif hi == 3:
                    load_xg(1, nc.sync)
                elif 8 <= hi < 14:
                    load_w2(0, hi - 8, nc.sync)
            sa01 = chain(gen_shared_sa(0, ("po", "po")),
                         gen_shared_sa(1, ("po", "po")))
            drive(gen_routed_a(0, filler_a0), sa01,
                  "MMMM" + "FM" * 8 + "MMMM")

            # S2: rB0 (streams rest of w2(e0) on scalar) interleaved with
            # SB0+SB1.  sync fillers: prefetch wa(e1) tiles for S3.
            def filler_b0(hi):
                if 2 <= hi < 10:
                    load_wa(1, hi - 2, nc.sync)
                elif hi == 10:
                    load_xs(2, nc.gpsimd)
            sb01 = chain(gen_shared_sb(0, ("p1", "p2")),
                         gen_shared_sb(1, ("p1", "p2")))
            drive(gen_routed_b(0, filler_b0, w2_eng=nc.scalar), sb01,
                  "MMF" * 8 + "M")

            # S3: rA1 (streams wa on sync) interleaved with SA2 then SB2.
            def filler_a1(hi):
                if hi == 4:
                    load_xs(3, nc.gpsimd)
                elif 8 <= hi < 14:
                    load_w2(1, hi - 8, nc.sync)
            sa_sb2 = chain(gen_shared_sa(2, ("po", "po")),
                           gen_shared_sb(2, ("po", "po")))
            drive(gen_routed_a(1, filler_a1), sa_sb2, "MMF" * 8)

            # S4: rB1 (streams rest of w2(e1) on sync) interleaved with
            # SA3 then SB3 (both on p1/p2).
            # SB3 runs at single-group granularity; the last two groups come
            # after the rB1 epilogue so their PE work covers the rout write
            # latency (shorter kernel tail).
            sa_sb3 = chain(gen_shared_sa(3, ("p1", "p2")),
                           gen_shared_sb(3, ("p1", "p2"), fine=True))
            drive(gen_routed_b(1), sa_sb3,
                  "MMF" * 4 + "MF" * 6 + "MMM" + "FF")

    nc.compile()
    return nc


def _route(xf, Wg):
    """Host router: returns (top-k expert ids, gates) per token."""
    logits = xf.astype(np.float64) @ Wg.astype(np.float64)        # [N, E]
    part = np.argpartition(-logits, K - 1, axis=1)[:, :K]          # [N, K]
    pl = np.take_along_axis(logits, part, axis=1)
    order = np.argsort(-pl, axis=1, kind="stable")
    topi = np.take_along_axis(part, order, axis=1)                 # [N, K] sorted
    tl = np.take_along_axis(logits, topi, axis=1)
    m = tl.max(axis=1, keepdims=True)
    e = np.exp(tl - m)
    gates = (e / e.sum(axis=1, keepdims=True)).astype(np.float32)  # [N, K]
    return topi, gates


def kernel(x, Wg, W1, b1, W11, b11, W2, b2, Ws1, bs1, Ws11, bs11, Ws2, bs2,
           _run_opts=None):
    xf = np.ascontiguousarray(x.reshape(N, D), dtype=np.float32)
    topi, gates = _route(xf, Wg)

    # token lists per expert
    flat_e = topi.reshape(-1)                        # [N*K]
    flat_tok = np.repeat(np.arange(N), K)
    flat_g = gates.reshape(-1)
    order = np.argsort(flat_e, kind="stable")
    counts = np.bincount(flat_e, minlength=E)
    starts = np.zeros(E + 1, np.int64)
    np.cumsum(counts, out=starts[1:])
    tok_sorted = flat_tok[order]
    g_sorted = flat_g[order]

    xs_arr = np.ascontiguousarray(
        xf.reshape(N, DT, P).transpose(2, 1, 0)).astype(ml_dtypes.bfloat16)

    in_maps = []
    meta = []          # (expert, idx, g) per (core, j)
    overflow = []      # (expert, idx, g) computed on host
    for c in range(NCORES):
        im = {}
        xg_arr = np.zeros((EPC, P, DT * C), ml_dtypes.bfloat16)
        gt_arr = np.zeros((P, EPC * CT), np.float32)
        wa_arr = np.empty((EPC, HT, P, 2 * DT * P), ml_dtypes.bfloat16)
        w2_arr = np.empty((EPC, HT, P, D), ml_dtypes.bfloat16)
        b1_arr = np.empty((P, EPC * HT), np.float32)
        b11_arr = np.empty((P, EPC * HT), np.float32)
        core_meta = []
        for j in range(EPC):
            e_id = c * EPC + j
            idx = tok_sorted[starts[e_id]:starts[e_id + 1]]
            g = g_sorted[starts[e_id]:starts[e_id + 1]]
            if len(idx) > C:
                overflow.append((e_id, idx[C:], g[C:]))
                idx, g = idx[:C], g[:C]
            n_e = len(idx)
            core_meta.append((e_id, idx, g))
            # gathered tokens, transposed: [P, DT, C]
            xpad = np.zeros((C, D), np.float32)
            xpad[:n_e] = xf[idx]
            xg_arr[j] = xpad.reshape(C, DT, P).transpose(2, 1, 0).reshape(
                P, DT * C).astype(ml_dtypes.bfloat16)
            gpad = np.zeros(CT * P, np.float32)
            gpad[:n_e] = g
            gt_arr[:, j * CT:(j + 1) * CT] = gpad.reshape(CT, P).T
            wa_arr[j, :, :, :DT * P] = np.asarray(W1[e_id]).reshape(
                DT, P, HT, P).transpose(2, 1, 0, 3).reshape(
                HT, P, DT * P).astype(ml_dtypes.bfloat16)
            wa_arr[j, :, :, DT * P:] = np.asarray(W11[e_id]).reshape(
                DT, P, HT, P).transpose(2, 1, 0, 3).reshape(
                HT, P, DT * P).astype(ml_dtypes.bfloat16)
            w2_arr[j] = np.asarray(W2[e_id]).reshape(
                HT, P, D).astype(ml_dtypes.bfloat16)
            b1_arr[:, j * HT:(j + 1) * HT] = \
                np.asarray(b1[e_id], np.float32).reshape(HT, P).T
            b11_arr[:, j * HT:(j + 1) * HT] = \
                np.asarray(b11[e_id], np.float32).reshape(HT, P).T
        meta.append(core_meta)
        im["xg"] = xg_arr
        im["gt"] = gt_arr
        im["wa"] = wa_arr
        im["w2"] = w2_arr
        im["b1"] = b1_arr
        im["b11"] = b11_arr
        # shared expert slice
        sl = slice(c * HSS, (c + 1) * HSS)
        im["xs"] = xs_arr
        wsa_arr = np.empty((HST, P, 2 * DT * P), ml_dtypes.bfloat16)
        wsa_arr[:, :, :DT * P] = np.asarray(Ws1)[:, sl].reshape(
            DT, P, HST, P).transpose(2, 1, 0, 3).reshape(
            HST, P, DT * P).astype(ml_dtypes.bfloat16)
        wsa_arr[:, :, DT * P:] = np.asarray(Ws11)[:, sl].reshape(
            DT, P, HST, P).transpose(2, 1, 0, 3).reshape(
            HST, P, DT * P).astype(ml_dtypes.bfloat16)
        im["wsa"] = wsa_arr
        im["ws2"] = np.asarray(Ws2)[sl].reshape(
            HST, P, D).astype(ml_dtypes.bfloat16)
        bsa_arr = np.empty((P, 2 * HST), np.float32)
        bsa_arr[:, :HST] = np.asarray(bs1, np.float32)[sl].reshape(HST, P).T
        bsa_arr[:, HST:] = np.asarray(bs11, np.float32)[sl].reshape(HST, P).T
        im["bsa"] = bsa_arr
        in_maps.append(im)

    if "nc" not in _CACHED:
        _CACHED["nc"] = _build_nc()
    nc = _CACHED["nc"]

    run_opts = _run_opts or {}
    res = bass_utils.run_bass_kernel_spmd(
        nc, in_maps, core_ids=list(range(NCORES)), **run_opts)
    _CACHED["last_results"] = res

    # ---- host-side unshard / combine ----
    y = np.zeros((N, D), np.float32)
    for c in range(NCORES):
        ro = np.asarray(res.results[c]["rout"], np.float32).reshape(
            EPC, CT * P, D)
        for j in range(EPC):
            e_id, idx, g = meta[c][j]
            n_e = len(idx)
            np.add.at(y, idx, ro[j, :n_e] + g[:, None] * b2[e_id][None, :])
        zc = np.asarray(res.results[c]["zout"], np.float32).reshape(N, D)
        if c == 0:
            z = zc
        else:
            z += zc

    for e_id, idx, g in overflow:
        xo = xf[idx]
        h = _silu(xo @ W1[e_id] + b1[e_id]) * (xo @ W11[e_id] + b11[e_id])
        np.add.at(y, idx, (h @ W2[e_id] + b2[e_id]) * g[:, None])

    out = y + z + np.asarray(bs2, np.float32)[None, :]
    return out.reshape(B, T, D).astype(np.float32)


def _silu(v):
    return v * (1.0 / (1.0 + np.exp(-v)))


# revision 50
# speedup vs baseline: 1.0037x; 1.0037x over previous
"""Trainium2 Bass kernel for a 16-expert top-2 MoE layer with shared expert.

Sharding (8 cores):
  - Routed experts: expert-parallel, 2 experts per core, host-side token
    dispatch (gather) with a static per-expert capacity of C tokens.
  - Shared expert: tensor-parallel over the 4096-wide intermediate dim
    (512 per core); partial outputs summed on host.
  - Router runs on host (tiny: [2048,1024]@[1024,16]).

All matmul operands run in bf16 (fp32 PSUM accumulation); end-to-end
relative error ~4e-3.  bf16 halves every weight stream vs the fp32r
baseline and enables fast-weight-load on all stationaries, which takes
DMA off the critical path: the kernel is PE-issue-bound at ~1 moving
column/cycle.

Program order interleaves shared-expert chunks between routed-expert
steps so the PE never idles while weights stream in, and weight streams
for the next phase are prefetched during the previous one (wA1 during
S2, w2(e0) during late S1) to avoid phase-boundary stalls.
Streamed routed weights go on the sync HWDGE queue; activations,
resident shared weights and the e0 w2 stream on the scalar HWDGE queue;
outputs on the gpsimd SWDGE queue.
"""
import sys
sys.path.insert(0, "/opt/trn_rl_repo")
import numpy as np
import ml_dtypes
from concourse import bacc, mybir
from concourse import tile
from concourse import bass_utils

# Problem shape (hardcoded per contract)
B, T, D = 2, 1024, 1024
N = B * T                # 2048 tokens
E = 16                   # routed experts
H = 2048                 # expert intermediate
K = 2                    # top-k
HS = 4096                # shared intermediate
NCORES = 8
EPC = E // NCORES        # experts per core = 2
HSS = HS // NCORES       # shared intermediate slice per core = 512

P = 128
C = 256                  # per-expert token capacity (mean load is 256)
CT = (C + P - 1) // P    # 2 token tiles per expert
DT = D // P              # 8 contraction slices over D
HT = H // P              # 16 H tiles
HST = HSS // P           # 4 shared-H tiles per core
SCH = 512                # shared-expert token chunk (moving dim)
NCH = N // SCH           # 4 chunks
DN = D // 512            # 2 output free tiles of 512

F32 = mybir.dt.float32
BF16 = mybir.dt.bfloat16
AF = mybir.ActivationFunctionType

_CACHED = {}


def _mtiles():
    """Token tiles of (offset, width) covering C in 128-row steps."""
    out = []
    off = 0
    while off < C:
        out.append((off, min(P, C - off)))
        off += P
    return out


def _build_nc():
    nc = bacc.Bacc("TRN2", target_bir_lowering=False, debug=False,
                   num_devices=NCORES)

    # ---- DRAM I/O (per-core) ----
    xg = nc.dram_tensor("xg", [EPC, P, DT * C], BF16, kind="ExternalInput").ap()
    gt = nc.dram_tensor("gt", [P, EPC * CT], F32, kind="ExternalInput").ap()
    wa = nc.dram_tensor("wa", [EPC, HT, P, 2 * DT * P], BF16,
                        kind="ExternalInput").ap()
    w2 = nc.dram_tensor("w2", [EPC, HT, P, D], BF16, kind="ExternalInput").ap()
    b1 = nc.dram_tensor("b1", [P, EPC * HT], F32, kind="ExternalInput").ap()
    b11 = nc.dram_tensor("b11", [P, EPC * HT], F32, kind="ExternalInput").ap()
    xs = nc.dram_tensor("xs", [P, DT, N], BF16, kind="ExternalInput").ap()
    wsa = nc.dram_tensor("wsa", [HST, P, 2 * DT * P], BF16,
                         kind="ExternalInput").ap()
    ws2 = nc.dram_tensor("ws2", [HST, P, D], BF16, kind="ExternalInput").ap()
    bsa = nc.dram_tensor("bsa", [P, 2 * HST], F32, kind="ExternalInput").ap()
    rout = nc.dram_tensor("rout", [EPC, CT, P, D], BF16,
                          kind="ExternalOutput").ap()
    zout = nc.dram_tensor("zout", [N // P, P, D], BF16,
                          kind="ExternalOutput").ap()

    mtiles = _mtiles()

    with tile.TileContext(nc) as tc:
        with tc.tile_pool(name="sb", bufs=1) as sb, \
             tc.tile_pool(name="ps", bufs=1, space="PSUM") as ps:

            xg_t = [None, None]

            def load_xg(j, eng, split=False):
                t = sb.tile([P, DT * C], BF16, tag=f"xg{j}", name=f"xg_t{j}")
                if split:
                    eng.dma_start(t[:, :DT * C // 2], xg[j, :, 0:DT * C // 2])
                    eng.dma_start(t[:, DT * C // 2:], xg[j, :, DT * C // 2:])
                else:
                    eng.dma_start(t[:], xg[j])
                xg_t[j] = t

            wa_t = {}

            def load_wa(j, hi, eng, split=False):
                t = sb.tile([P, 2 * DT * P], BF16, tag="wa", bufs=12,
                            name=f"wa_t{j}_{hi}")
                if split:
                    eng.dma_start(t[:, :DT * P], wa[j, hi, :, 0:DT * P])
                    eng.dma_start(t[:, DT * P:], wa[j, hi, :, DT * P:])
                else:
                    eng.dma_start(t[:], wa[j, hi])
                wa_t[(j, hi)] = t

            w2_t = {}

            def load_w2(j, hi, eng):
                t = sb.tile([P, D], BF16, tag="w2", bufs=8,
                            name=f"w2_t{j}_{hi}")
                eng.dma_start(t[:], w2[j, hi])
                w2_t[(j, hi)] = t

            wsa_t = [None] * HST
            ws2_t = [None] * HST

            def load_wsa(hi, eng):
                t = sb.tile([P, 2 * DT * P], BF16, tag=f"wsa_{hi}",
                            name=f"wsa_t{hi}")
                eng.dma_start(t[:], wsa[hi])
                wsa_t[hi] = t

            def load_ws2(hi, eng):
                t = sb.tile([P, D], BF16, tag=f"ws2_{hi}", name=f"ws2_t{hi}")
                eng.dma_start(t[:], ws2[hi])
                ws2_t[hi] = t

            xs_tiles = {}

            def load_xs(t, eng, split=1):
                xs_t = sb.tile([P, DT * SCH], BF16, tag="xs", bufs=2,
                               name=f"xs_t{t}")
                qd = DT // split
                for q in range(split):
                    eng.dma_start(
                        xs_t[:, q * qd * SCH:(q + 1) * qd * SCH].rearrange(
                            "p (ds s) -> p ds s", ds=qd),
                        xs[:, q * qd:(q + 1) * qd, t * SCH:(t + 1) * SCH])
                xs_tiles[t] = xs_t

            h_t = {0: [], 1: []}
            s_t = {}

            def gen_routed_a(j, dma_filler=None):
                """16 steps, one H-tile each: stream wa, 2 psum groups,
                silu/bias-add/mul -> hT[hi] (bf16)."""
                for hi in range(HT):
                    if (j, hi) not in wa_t:
                        load_wa(j, hi, nc.sync)
                    if dma_filler is not None:
                        dma_filler(hi)
                    wt = wa_t[(j, hi)]
                    ps1 = ps.tile([P, 512], F32, tag="p1", bufs=2,
                                  name=f"ps1_{j}_{hi}")
                    ps2 = ps.tile([P, 512], F32, tag="p2", bufs=2,
                                  name=f"ps2_{j}_{hi}")
                    if j == 0 and hi == 0:
                        # first step: emit in DMA piece-arrival order
                        # (w1 half, xg half 1, w11 half, xg half 2)
                        halves = [(ps1, 0, range(0, DT // 2)),
                                  (ps2, DT * P, range(0, DT // 2)),
                                  (ps1, 0, range(DT // 2, DT)),
                                  (ps2, DT * P, range(DT // 2, DT))]
                        for pst, woff, dss in halves:
                            for ds in dss:
                                nc.tensor.matmul(
                                    pst[:, :C],
                                    wt[:, woff + ds * P:woff + (ds + 1) * P],
                                    xg_t[j][:, ds * C:(ds + 1) * C],
                                    start=(ds == 0), stop=(ds == DT - 1))
                    else:
                        for ds in range(DT):
                            nc.tensor.matmul(
                                ps1[:, :C],
                                wt[:, ds * P:(ds + 1) * P],
                                xg_t[j][:, ds * C:(ds + 1) * C],
                                start=(ds == 0), stop=(ds == DT - 1))
                        for ds in range(DT):
                            nc.tensor.matmul(
                                ps2[:, :C],
                                wt[:, DT * P + ds * P:DT * P + (ds + 1) * P],
                                xg_t[j][:, ds * C:(ds + 1) * C],
                                start=(ds == 0), stop=(ds == DT - 1))
                    t1 = sb.tile([P, C], F32, tag="t1r", bufs=2,
                                 name=f"t1_{j}_{hi}")
                    nc.scalar.activation(t1[:], ps1[:, :C], AF.Silu,
                                         bias=b1_all[:, j * HT + hi:j * HT + hi + 1])
                    t2 = sb.tile([P, C], F32, tag="t2r", bufs=2,
                                 name=f"t2_{j}_{hi}")
                    nc.vector.tensor_scalar_add(
                        t2[:], ps2[:, :C],
                        b11_all[:, j * HT + hi:j * HT + hi + 1])
                    ht = sb.tile([P, C], BF16, tag=f"h_{hi}", name=f"h_{j}_{hi}")
                    nc.vector.tensor_mul(ht[:], t1[:], t2[:])
                    h_t[j].append(ht)
                    yield

            def gen_routed_b(j, dma_filler=None, w2_eng=None):
                """16 steps (one per streamed w2 tile) + epilogue step."""
                pos = [[ps.tile([P, 512], F32, tag="po", bufs=4,
                                name=f"po_{j}_{m}_{n2}")
                        for n2 in range(DN)] for m in range(len(mtiles))]
                for hi in range(HT):
                    if (j, hi) not in w2_t:
                        load_w2(j, hi, w2_eng or nc.sync)
                    if dma_filler is not None:
                        dma_filler(hi)
                    w2t = w2_t[(j, hi)]
                    for m, (off, mw) in enumerate(mtiles):
                        for n2 in range(DN):
                            nc.tensor.matmul(
                                pos[m][n2][:mw, :],
                                h_t[j][hi][:, off:off + mw],
                                w2t[:, n2 * 512:(n2 + 1) * 512],
                                start=(hi == 0), stop=(hi == HT - 1))
                    yield
                spread = j == EPC - 1   # final expert: minimize kernel tail
                for i, (m, n2) in enumerate(
                        (m, n2) for m in range(len(mtiles)) for n2 in range(DN)):
                    off, mw = mtiles[m]
                    o_t = sb.tile([P, 512], BF16, tag="ot", bufs=4,
                                  name=f"o_t{j}_{m}_{n2}")
                    gsc = g_all[:mw, j * CT + m:j * CT + m + 1]
                    if spread and i >= 2:
                        nc.vector.tensor_scalar_mul(
                            o_t[:mw, :], pos[m][n2][:mw, :], gsc)
                    else:
                        nc.scalar.activation(
                            o_t[:mw, :], pos[m][n2][:mw, :], AF.Identity,
                            scale=gsc)
                    oe = nc.gpsimd
                    if spread:
                        oe = nc.sync if i < 2 else nc.gpsimd
                    oe.dma_start(
                        rout[j, m, 0:mw, n2 * 512:(n2 + 1) * 512],
                        o_t[:mw, :])
                yield

            def gen_shared_sa(t, p_tags, dma_filler=None):
                """4 steps, one shared-H tile each."""
                s_t[t] = []
                for hi in range(HST):
                    if dma_filler is not None:
                        dma_filler(hi)
                    ps1 = ps.tile([P, 512], F32, tag=p_tags[0],
                                  bufs=4 if p_tags[0] == "po" else 2,
                                  name=f"sps1_{t}_{hi}")
                    ps2 = ps.tile([P, 512], F32, tag=p_tags[1],
                                  bufs=4 if p_tags[1] == "po" else 2,
                                  name=f"sps2_{t}_{hi}")
                    if t == 0 and hi == 0:
                        # first step: emit in DMA piece-arrival order
                        # (wsa h1, xs h1, wsa h2, xs h2)
                        halves = [(ps1, 0, range(0, DT // 2)),
                                  (ps2, DT * P, range(0, DT // 2)),
                                  (ps1, 0, range(DT // 2, DT)),
                                  (ps2, DT * P, range(DT // 2, DT))]
                        for pst, woff, dss in halves:
                            for ds in dss:
                                nc.tensor.matmul(
                                    pst[:],
                                    wsa_t[hi][:, woff + ds * P:
                                              woff + (ds + 1) * P],
                                    xs_tiles[t][:, ds * SCH:(ds + 1) * SCH],
                                    start=(ds == 0), stop=(ds == DT - 1))
                    else:
                        for ds in range(DT):
                            nc.tensor.matmul(
                                ps1[:],
                                wsa_t[hi][:, ds * P:(ds + 1) * P],
                                xs_tiles[t][:, ds * SCH:(ds + 1) * SCH],
                                start=(ds == 0), stop=(ds == DT - 1))
                        for ds in range(DT):
                            nc.tensor.matmul(
                                ps2[:],
                                wsa_t[hi][:, DT * P + ds * P:
                                          DT * P + (ds + 1) * P],
                                xs_tiles[t][:, ds * SCH:(ds + 1) * SCH],
                                start=(ds == 0), stop=(ds == DT - 1))
                    t1 = sb.tile([P, 512], F32, tag="t1", bufs=2,
                                 name=f"st1_{t}_{hi}")
                    nc.scalar.activation(t1[:], ps1[:], AF.Silu,
                                         bias=bs_all[:, hi:hi + 1])
                    t2 = sb.tile([P, 512], F32, tag="t2", bufs=2,
                                 name=f"st2_{t}_{hi}")
                    nc.vector.tensor_scalar_add(t2[:], ps2[:],
                                                bs_all[:, HST + hi:HST + hi + 1])
                    st = sb.tile([P, 512], BF16, tag=f"s_{hi}", bufs=2,
                                 name=f"s_{t}_{hi}")
                    nc.vector.tensor_mul(st[:], t1[:], t2[:])
                    s_t[t].append(st)
                    yield

            def gen_shared_sb(t, p_tags, fine=False):
                """(token-tile, D-half) groups; 2 per step, or 1 if fine."""
                groups = [(mm, n2) for mm in range(SCH // P) for n2 in range(DN)]
                step = 1 if fine else 2
                for k in range(0, len(groups), step):
                    for gi, (mm, n2) in enumerate(groups[k:k + step], k):
                        _tg = p_tags[gi // 2 % 2]
                        po = ps.tile([P, 512], F32, tag=_tg,
                                     bufs=4 if _tg == "po" else 2,
                                     name=f"spo_{t}_{mm}_{n2}")
                        for hi in range(HST):
                            nc.tensor.matmul(
                                po[:],
                                s_t[t][hi][:, mm * P:(mm + 1) * P],
                                ws2_t[hi][:, n2 * 512:(n2 + 1) * 512],
                                start=(hi == 0), stop=(hi == HST - 1))
                        z_t = sb.tile([P, 512], BF16, tag="zt", bufs=4,
                                      name=f"z_t{t}_{mm}_{n2}")
                        if gi % 2:
                            nc.vector.tensor_copy(z_t[:], po[:])
                        else:
                            nc.scalar.activation(z_t[:], po[:], AF.Identity)
                        z_eng = nc.sync if fine and gi == 6 else nc.gpsimd
                        z_eng.dma_start(
                            zout[t * (SCH // P) + mm, :, n2 * 512:(n2 + 1) * 512],
                            z_t[:])
                    yield

            def drive(main_gen, fill_gen, sched):
                """Consume steps per explicit schedule string, then drain both."""
                for ch in sched:
                    g = main_gen if ch == "M" else fill_gen
                    try:
                        next(g)
                    except StopIteration:
                        pass
                for g in (main_gen, fill_gen):
                    while True:
                        try:
                            next(g)
                        except StopIteration:
                            break

            def chain(*gens):
                for g in gens:
                    yield from g

            # --- prologue: shared-expert work starts first (it needs only
            # ~145 GB/s of just-in-time DMA vs ~290 for routed), so the PE
            # fills the DMA-constrained bring-up window while the 8MB wa0
            # stream prefetches in the background. Critical pieces go on
            # sync, interleaved in the order SA0 step 0 consumes them. ---
            wsa0 = sb.tile([P, 2 * DT * P], BF16, tag="wsa_0", name="wsa_t0")
            xs0t = sb.tile([P, DT * SCH], BF16, tag="xs", bufs=2,
                           name="xs_t0")
            hx = DT // 2
            nc.sync.dma_start(wsa0[:, :DT * P], wsa[0, :, 0:DT * P])
            nc.sync.dma_start(
                xs0t[:, :hx * SCH].rearrange("p (ds s) -> p ds s", ds=hx),
                xs[:, 0:hx, 0:SCH])
            nc.sync.dma_start(wsa0[:, DT * P:], wsa[0, :, DT * P:])
            nc.sync.dma_start(
                xs0t[:, hx * SCH:].rearrange("p (ds s) -> p ds s", ds=hx),
                xs[:, hx:DT, 0:SCH])
            wsa_t[0] = wsa0
            xs_tiles[0] = xs0t
            # scalar: biases, remaining shared residents
            bs_all = sb.tile([P, 2 * HST], F32, name="bs_all")
            nc.scalar.dma_start(bs_all[:], bsa[:])
            load_wsa(1, nc.scalar)
            b1_all = sb.tile([P, EPC * HT], F32, name="b1_all")
            nc.scalar.dma_start(b1_all[:], b1[:])
            b11_all = sb.tile([P, EPC * HT], F32, name="b11_all")
            nc.scalar.dma_start(b11_all[:], b11[:])
            load_wsa(2, nc.scalar)
            load_wsa(3, nc.scalar)
            g_all = sb.tile([P, EPC * CT], F32, name="g_all")
            nc.scalar.dma_start(g_all[:], gt[:])

            # S1a: SA0+SA1 run alone; fillers prefetch the rA0 stream
            # (wa tiles + xg0 on sync) and later shared inputs.
            def filler_sa0(hi):
                if hi == 1:
                    load_wa(0, 0, nc.sync)
                    load_wa(0, 1, nc.sync)
                elif hi == 2:
                    load_wa(0, 2, nc.sync)
                    load_xs(1, nc.scalar)
                elif hi == 3:
                    load_wa(0, 3, nc.sync)
                    load_xg(0, nc.sync)

            def filler_sa1(hi):
                if hi == 0:
                    load_wa(0, 4, nc.sync)
                    load_wa(0, 5, nc.sync)
                elif hi == 1:
                    load_wa(0, 6, nc.sync)
                    load_wa(0, 7, nc.sync)
                elif hi == 2:
                    load_wa(0, 8, nc.sync)
                    load_wa(0, 9, nc.sync)
                    load_ws2(0, nc.scalar)
                    load_ws2(1, nc.scalar)
                elif hi == 3:
                    load_wa(0, 10, nc.sync)
                    load_wa(0, 11, nc.sync)
                    load_ws2(2, nc.scalar)
                    load_ws2(3, nc.scalar)

            for _ in chain(gen_shared_sa(0, ("po", "po"), filler_sa0),
                           gen_shared_sa(1, ("po", "po"), filler_sa1)):
                pass

            # S1b: rA0 runs alone (wa mostly prefetched; tail JIT), with
            # w2(e0) prefetch for S2 and xg1 for S3.
            def filler_a0(hi):
                if hi == 3:
                    load_xg(1, nc.sync)
                elif 8 <= hi < 14:
                    load_w2(0, hi - 8, nc.sync)
            for _ in gen_routed_a(0, filler_a0):
                pass

            # S2: rB0 (streams rest of w2(e0) on scalar) interleaved with
            # SB0+SB1.  sync fillers: prefetch wa(e1) tiles for S3.
            def filler_b0(hi):
                if 2 <= hi < 10:
                    load_wa(1, hi - 2, nc.sync)
                elif hi == 10:
                    load_xs(2, nc.gpsimd)
            sb01 = chain(gen_shared_sb(0, ("p1", "p2")),
                         gen_shared_sb(1, ("p1", "p2")))
            drive(gen_routed_b(0, filler_b0, w2_eng=nc.scalar), sb01,
                  "MMF" * 8 + "M")

            # S3: rA1 (streams wa on sync) interleaved with SA2 then SB2.
            def filler_a1(hi):
                if hi == 4:
                    load_xs(3, nc.gpsimd)
                elif 8 <= hi < 14:
                    load_w2(1, hi - 8, nc.sync)
            sa_sb2 = chain(gen_shared_sa(2, ("po", "po")),
                           gen_shared_sb(2, ("po", "po")))
            drive(gen_routed_a(1, filler_a1), sa_sb2, "MMF" * 8)

            # S4: rB1 (streams rest of w2(e1) on sync) interleaved with
            # SA3 then SB3 (both on p1/p2).
            # SB3 runs at single-group granularity; the last two groups come
            # after the rB1 epilogue so their PE work covers the rout write
            # latency (shorter kernel tail).
            sa_sb3 = chain(gen_shared_sa(3, ("p1", "p2")),
                           gen_shared_sb(3, ("p1", "p2"), fine=True))
            drive(gen_routed_b(1), sa_sb3,
                  "MMF" * 4 + "MF" * 6 + "MMM" + "FF")

    nc.compile()
    return nc


def _route(xf, Wg):
    """Host router: returns (top-k expert ids, gates) per token."""
    logits = xf.astype(np.float64) @ Wg.astype(np.float64)        # [N, E]
    part = np.argpartition(-logits, K - 1, axis=1)[:, :K]          # [N, K]
    pl = np.take_along_axis(logits, part, axis=1)
    order = np.argsort(-pl, axis=1, kind="stable")
    topi = np.take_along_axis(part, order, axis=1)                 # [N, K] sorted
    tl = np.take_along_axis(logits, topi, axis=1)
    m = tl.max(axis=1, keepdims=True)
    e = np.exp(tl - m)
    gates = (e / e.sum(axis=1, keepdims=True)).astype(np.float32)  # [N, K]
    return topi, gates


def kernel(x, Wg, W1, b1, W11, b11, W2, b2, Ws1, bs1, Ws11, bs11, Ws2, bs2,
           _run_opts=None):
    xf = np.ascontiguousarray(x.reshape(N, D), dtype=np.float32)
    topi, gates = _route(xf, Wg)

    # token lists per expert
    flat_e = topi.reshape(-1)                        # [N*K]
    flat_tok = np.repeat(np.arange(N), K)
    flat_g = gates.reshape(-1)
    order = np.argsort(flat_e, kind="stable")
    counts = np.bincount(flat_e, minlength=E)
    starts = np.zeros(E + 1, np.int64)
    np.cumsum(counts, out=starts[1:])
    tok_sorted = flat_tok[order]
    g_sorted = flat_g[order]

    xs_arr = np.ascontiguousarray(
        xf.reshape(N, DT, P).transpose(2, 1, 0)).astype(ml_dtypes.bfloat16)

    in_maps = []
    meta = []          # (expert, idx, g) per (core, j)
    overflow = []      # (expert, idx, g) computed on host
    for c in range(NCORES):
        im = {}
        xg_arr = np.zeros((EPC, P, DT * C), ml_dtypes.bfloat16)
        gt_arr = np.zeros((P, EPC * CT), np.float32)
        wa_arr = np.empty((EPC, HT, P, 2 * DT * P), ml_dtypes.bfloat16)
        w2_arr = np.empty((EPC, HT, P, D), ml_dtypes.bfloat16)
        b1_arr = np.empty((P, EPC * HT), np.float32)
        b11_arr = np.empty((P, EPC * HT), np.float32)
        core_meta = []
        for j in range(EPC):
            e_id = c * EPC + j
            idx = tok_sorted[starts[e_id]:starts[e_id + 1]]
            g = g_sorted[starts[e_id]:starts[e_id + 1]]
            if len(idx) > C:
                overflow.append((e_id, idx[C:], g[C:]))
                idx, g = idx[:C], g[:C]
            n_e = len(idx)
            core_meta.append((e_id, idx, g))
            # gathered tokens, transposed: [P, DT, C]
            xpad = np.zeros((C, D), np.float32)
            xpad[:n_e] = xf[idx]
            xg_arr[j] = xpad.reshape(C, DT, P).transpose(2, 1, 0).reshape(
                P, DT * C).astype(ml_dtypes.bfloat16)
            gpad = np.zeros(CT * P, np.float32)
            gpad[:n_e] = g
            gt_arr[:, j * CT:(j + 1) * CT] = gpad.reshape(CT, P).T
            wa_arr[j, :, :, :DT * P] = np.asarray(W1[e_id]).reshape(
                DT, P, HT, P).transpose(2, 1, 0, 3).reshape(
                HT, P, DT * P).astype(ml_dtypes.bfloat16)
            wa_arr[j, :, :, DT * P:] = np.asarray(W11[e_id]).reshape(
                DT, P, HT, P).transpose(2, 1, 0, 3).reshape(
                HT, P, DT * P).astype(ml_dtypes.bfloat16)
            w2_arr[j] = np.asarray(W2[e_id]).reshape(
                HT, P, D).astype(ml_dtypes.bfloat16)
            b1_arr[:, j * HT:(j + 1) * HT] = \
                np.asarray(b1[e_id], np.float32).reshape(HT, P).T
            b11_arr[:, j * HT:(j + 1) * HT] = \
                np.asarray(b11[e_id], np.float32).reshape(HT, P).T
        meta.append(core_meta)
        im["xg"] = xg_arr
        im["gt"] = gt_arr
        im["wa"] = wa_arr
        im["w2"] = w2_arr
        im["b1"] = b1_arr
        im["b11"] = b11_arr
        # shared expert slice
        sl = slice(c * HSS, (c + 1) * HSS)
        im["xs"] = xs_arr
        wsa_arr = np.empty((HST, P, 2 * DT * P), ml_dtypes.bfloat16)
        wsa_arr[:, :, :DT * P] = np.asarray(Ws1)[:, sl].reshape(
            DT, P, HST, P).transpose(2, 1, 0, 3).reshape(
            HST, P, DT * P).astype(ml_dtypes.bfloat16)
        wsa_arr[:, :, DT * P:] = np.asarray(Ws11)[:, sl].reshape(
            DT, P, HST, P).transpose(2, 1, 0, 3).reshape(
            HST, P, DT * P).astype(ml_dtypes.bfloat16)
        im["wsa"] = wsa_arr
        im["ws2"] = np.asarray(Ws2)[sl].reshape(
            HST, P, D).astype(ml_dtypes.bfloat16)
        bsa_arr = np.empty((P, 2 * HST), np.float32)
        bsa_arr[:, :HST] = np.asarray(bs1, np.float32)[sl].reshape(HST, P).T
        bsa_arr[:, HST:] = np.asarray(bs11, np.float32)[sl].reshape(HST, P).T
        im["bsa"] = bsa_arr
        in_maps.append(im)

    if "nc" not in _CACHED:
        _CACHED["nc"] = _build_nc()
    nc = _CACHED["nc"]

    run_opts = _run_opts or {}
    res = bass_utils.run_bass_kernel_spmd(
        nc, in_maps, core_ids=list(range(NCORES)), **run_opts)
    _CACHED["last_results"] = res

    # ---- host-side unshard / combine ----
    y = np.zeros((N, D), np.float32)
    for c in range(NCORES):
        ro = np.asarray(res.results[c]["rout"], np.float32).reshape(
            EPC, CT * P, D)
        for j in range(EPC):
            e_id, idx, g = meta[c][j]
            n_e = len(idx)
            np.add.at(y, idx, ro[j, :n_e] + g[:, None] * b2[e_id][None, :])
        zc = np.asarray(res.results[c]["zout"], np.float32).reshape(N, D)
        if c == 0:
            z = zc
        else:
            z += zc

    for e_id, idx, g in overflow:
        xo = xf[idx]
        h = _silu(xo @ W1[e_id] + b1[e_id]) * (xo @ W11[e_id] + b11[e_id])
        np.add.at(y, idx, (h @ W2[e_id] + b2[e_id]) * g[:, None])

    out = y + z + np.asarray(bs2, np.float32)[None, :]
    return out.reshape(B, T, D).astype(np.float32)


def _silu(v):
    return v * (1.0 / (1.0 + np.exp(-v)))


# revision 55
# speedup vs baseline: 1.0057x; 1.0019x over previous
"""Trainium2 Bass kernel for a 16-expert top-2 MoE layer with shared expert.

Sharding (8 cores):
  - Routed experts: expert-parallel, 2 experts per core, host-side token
    dispatch (gather) with a static per-expert capacity of C tokens.
  - Shared expert: tensor-parallel over the 4096-wide intermediate dim
    (512 per core); partial outputs summed on host.
  - Router runs on host (tiny: [2048,1024]@[1024,16]).

All matmul operands run in bf16 (fp32 PSUM accumulation); end-to-end
relative error ~4e-3.  bf16 halves every weight stream vs the fp32r
baseline and enables fast-weight-load on all stationaries, which takes
DMA off the critical path: the kernel is PE-issue-bound at ~1 moving
column/cycle.

Program order interleaves shared-expert chunks between routed-expert
steps so the PE never idles while weights stream in, and weight streams
for the next phase are prefetched during the previous one (wA1 during
S2, w2(e0) during late S1) to avoid phase-boundary stalls.
Streamed routed weights go on the sync HWDGE queue; activations,
resident shared weights and the e0 w2 stream on the scalar HWDGE queue;
outputs on the gpsimd SWDGE queue.
"""
import sys
sys.path.insert(0, "/opt/trn_rl_repo")
import numpy as np
import ml_dtypes
from concourse import bacc, mybir
from concourse import tile
from concourse import bass_utils

# Problem shape (hardcoded per contract)
B, T, D = 2, 1024, 1024
N = B * T                # 2048 tokens
E = 16                   # routed experts
H = 2048                 # expert intermediate
K = 2                    # top-k
HS = 4096                # shared intermediate
NCORES = 8
EPC = E // NCORES        # experts per core = 2
HSS = HS // NCORES       # shared intermediate slice per core = 512

P = 128
C = 256                  # per-expert token capacity (mean load is 256)
CT = (C + P - 1) // P    # 2 token tiles per expert
DT = D // P              # 8 contraction slices over D
HT = H // P              # 16 H tiles
HST = HSS // P           # 4 shared-H tiles per core
SCH = 512                # shared-expert token chunk (moving dim)
NCH = N // SCH           # 4 chunks
DN = D // 512            # 2 output free tiles of 512

F32 = mybir.dt.float32
BF16 = mybir.dt.bfloat16
AF = mybir.ActivationFunctionType

_CACHED = {}


def _mtiles():
    """Token tiles of (offset, width) covering C in 128-row steps."""
    out = []
    off = 0
    while off < C:
        out.append((off, min(P, C - off)))
        off += P
    return out


def _build_nc():
    nc = bacc.Bacc("TRN2", target_bir_lowering=False, debug=False,
                   num_devices=NCORES)

    # ---- DRAM I/O (per-core) ----
    xg = nc.dram_tensor("xg", [EPC, P, DT * C], BF16, kind="ExternalInput").ap()
    gt = nc.dram_tensor("gt", [P, EPC * CT], F32, kind="ExternalInput").ap()
    wa = nc.dram_tensor("wa", [EPC, HT, P, 2 * DT * P], BF16,
                        kind="ExternalInput").ap()
    w2 = nc.dram_tensor("w2", [EPC, HT, P, D], BF16, kind="ExternalInput").ap()
    b1 = nc.dram_tensor("b1", [P, EPC * HT], F32, kind="ExternalInput").ap()
    b11 = nc.dram_tensor("b11", [P, EPC * HT], F32, kind="ExternalInput").ap()
    xs = nc.dram_tensor("xs", [P, NCH, DT * SCH], BF16,
                        kind="ExternalInput").ap()
    wsa = nc.dram_tensor("wsa", [HST, P, 2 * DT * P], BF16,
                         kind="ExternalInput").ap()
    ws2 = nc.dram_tensor("ws2", [HST, P, D], BF16, kind="ExternalInput").ap()
    bsa = nc.dram_tensor("bsa", [P, 2 * HST], F32, kind="ExternalInput").ap()
    rout = nc.dram_tensor("rout", [EPC, CT, P, D], BF16,
                          kind="ExternalOutput").ap()
    zout = nc.dram_tensor("zout", [N // P, P, D], BF16,
                          kind="ExternalOutput").ap()

    mtiles = _mtiles()

    with tile.TileContext(nc) as tc:
        with tc.tile_pool(name="sb", bufs=1) as sb, \
             tc.tile_pool(name="ps", bufs=1, space="PSUM") as ps:

            xg_t = [None, None]

            def load_xg(j, eng, split=False):
                t = sb.tile([P, DT * C], BF16, tag=f"xg{j}", name=f"xg_t{j}")
                if split:
                    eng.dma_start(t[:, :DT * C // 2], xg[j, :, 0:DT * C // 2])
                    eng.dma_start(t[:, DT * C // 2:], xg[j, :, DT * C // 2:])
                else:
                    eng.dma_start(t[:], xg[j])
                xg_t[j] = t

            wa_t = {}

            def load_wa(j, hi, eng, split=False):
                t = sb.tile([P, 2 * DT * P], BF16, tag="wa", bufs=12,
                            name=f"wa_t{j}_{hi}")
                if split:
                    eng.dma_start(t[:, :DT * P], wa[j, hi, :, 0:DT * P])
                    eng.dma_start(t[:, DT * P:], wa[j, hi, :, DT * P:])
                else:
                    eng.dma_start(t[:], wa[j, hi])
                wa_t[(j, hi)] = t

            w2_t = {}

            def load_w2(j, hi, eng):
                t = sb.tile([P, D], BF16, tag="w2", bufs=8,
                            name=f"w2_t{j}_{hi}")
                eng.dma_start(t[:], w2[j, hi])
                w2_t[(j, hi)] = t

            wsa_t = [None] * HST
            ws2_t = [None] * HST

            def load_wsa(hi, eng):
                t = sb.tile([P, 2 * DT * P], BF16, tag=f"wsa_{hi}",
                            name=f"wsa_t{hi}")
                eng.dma_start(t[:], wsa[hi])
                wsa_t[hi] = t

            def load_ws2(hi, eng):
                t = sb.tile([P, D], BF16, tag=f"ws2_{hi}", name=f"ws2_t{hi}")
                eng.dma_start(t[:], ws2[hi])
                ws2_t[hi] = t

            xs_tiles = {}

            def load_xs(t, eng, split=1):
                xs_t = sb.tile([P, DT * SCH], BF16, tag="xs", bufs=2,
                               name=f"xs_t{t}")
                qw = DT * SCH // split
                for q in range(split):
                    eng.dma_start(xs_t[:, q * qw:(q + 1) * qw],
                                  xs[:, t, q * qw:(q + 1) * qw])
                xs_tiles[t] = xs_t

            h_t = {0: [], 1: []}
            s_t = {}

            def gen_routed_a(j, dma_filler=None):
                """16 steps, one H-tile each: stream wa, 2 psum groups,
                silu/bias-add/mul -> hT[hi] (bf16)."""
                for hi in range(HT):
                    if (j, hi) not in wa_t:
                        load_wa(j, hi, nc.sync)
                    if dma_filler is not None:
                        dma_filler(hi)
                    wt = wa_t[(j, hi)]
                    ps1 = ps.tile([P, 512], F32, tag="p1", bufs=2,
                                  name=f"ps1_{j}_{hi}")
                    ps2 = ps.tile([P, 512], F32, tag="p2", bufs=2,
                                  name=f"ps2_{j}_{hi}")
                    if j == 0 and hi == 0:
                        # first step: emit in DMA piece-arrival order
                        # (w1 half, xg half 1, w11 half, xg half 2)
                        halves = [(ps1, 0, range(0, DT // 2)),
                                  (ps2, DT * P, range(0, DT // 2)),
                                  (ps1, 0, range(DT // 2, DT)),
                                  (ps2, DT * P, range(DT // 2, DT))]
                        for pst, woff, dss in halves:
                            for ds in dss:
                                nc.tensor.matmul(
                                    pst[:, :C],
                                    wt[:, woff + ds * P:woff + (ds + 1) * P],
                                    xg_t[j][:, ds * C:(ds + 1) * C],
                                    start=(ds == 0), stop=(ds == DT - 1))
                    else:
                        for ds in range(DT):
                            nc.tensor.matmul(
                                ps1[:, :C],
                                wt[:, ds * P:(ds + 1) * P],
                                xg_t[j][:, ds * C:(ds + 1) * C],
                                start=(ds == 0), stop=(ds == DT - 1))
                        for ds in range(DT):
                            nc.tensor.matmul(
                                ps2[:, :C],
                                wt[:, DT * P + ds * P:DT * P + (ds + 1) * P],
                                xg_t[j][:, ds * C:(ds + 1) * C],
                                start=(ds == 0), stop=(ds == DT - 1))
                    t1 = sb.tile([P, C], F32, tag="t1r", bufs=2,
                                 name=f"t1_{j}_{hi}")
                    nc.scalar.activation(t1[:], ps1[:, :C], AF.Silu,
                                         bias=b1_all[:, j * HT + hi:j * HT + hi + 1])
                    t2 = sb.tile([P, C], F32, tag="t2r", bufs=2,
                                 name=f"t2_{j}_{hi}")
                    nc.vector.tensor_scalar_add(
                        t2[:], ps2[:, :C],
                        b11_all[:, j * HT + hi:j * HT + hi + 1])
                    ht = sb.tile([P, C], BF16, tag=f"h_{hi}", name=f"h_{j}_{hi}")
                    nc.vector.tensor_mul(ht[:], t1[:], t2[:])
                    h_t[j].append(ht)
                    yield

            def gen_routed_b(j, dma_filler=None, w2_eng=None):
                """16 steps (one per streamed w2 tile) + epilogue step."""
                pos = [[ps.tile([P, 512], F32, tag="po", bufs=4,
                                name=f"po_{j}_{m}_{n2}")
                        for n2 in range(DN)] for m in range(len(mtiles))]
                for hi in range(HT):
                    if (j, hi) not in w2_t:
                        load_w2(j, hi, w2_eng or nc.sync)
                    if dma_filler is not None:
                        dma_filler(hi)
                    w2t = w2_t[(j, hi)]
                    for m, (off, mw) in enumerate(mtiles):
                        for n2 in range(DN):
                            nc.tensor.matmul(
                                pos[m][n2][:mw, :],
                                h_t[j][hi][:, off:off + mw],
                                w2t[:, n2 * 512:(n2 + 1) * 512],
                                start=(hi == 0), stop=(hi == HT - 1))
                    yield
                spread = j == EPC - 1   # final expert: minimize kernel tail
                for i, (m, n2) in enumerate(
                        (m, n2) for m in range(len(mtiles)) for n2 in range(DN)):
                    off, mw = mtiles[m]
                    o_t = sb.tile([P, 512], BF16, tag="ot", bufs=4,
                                  name=f"o_t{j}_{m}_{n2}")
                    gsc = g_all[:mw, j * CT + m:j * CT + m + 1]
                    if spread and i >= 2:
                        nc.vector.tensor_scalar_mul(
                            o_t[:mw, :], pos[m][n2][:mw, :], gsc)
                    else:
                        nc.scalar.activation(
                            o_t[:mw, :], pos[m][n2][:mw, :], AF.Identity,
                            scale=gsc)
                    oe = nc.gpsimd
                    if spread:
                        oe = nc.sync if i < 2 else nc.gpsimd
                    oe.dma_start(
                        rout[j, m, 0:mw, n2 * 512:(n2 + 1) * 512],
                        o_t[:mw, :])
                yield

            def gen_shared_sa(t, p_tags, dma_filler=None):
                """4 steps, one shared-H tile each."""
                s_t[t] = []
                for hi in range(HST):
                    if dma_filler is not None:
                        dma_filler(hi)
                    ps1 = ps.tile([P, 512], F32, tag=p_tags[0],
                                  bufs=4 if p_tags[0] == "po" else 2,
                                  name=f"sps1_{t}_{hi}")
                    ps2 = ps.tile([P, 512], F32, tag=p_tags[1],
                                  bufs=4 if p_tags[1] == "po" else 2,
                                  name=f"sps2_{t}_{hi}")
                    if t == 0 and hi == 0:
                        # first step: emit in DMA piece-arrival order
                        # (wsa h1, xs h1, wsa h2, xs h2)
                        halves = [(ps1, 0, range(0, DT // 2)),
                                  (ps2, DT * P, range(0, DT // 2)),
                                  (ps1, 0, range(DT // 2, DT)),
                                  (ps2, DT * P, range(DT // 2, DT))]
                        for pst, woff, dss in halves:
                            for ds in dss:
                                nc.tensor.matmul(
                                    pst[:],
                                    wsa_t[hi][:, woff + ds * P:
                                              woff + (ds + 1) * P],
                                    xs_tiles[t][:, ds * SCH:(ds + 1) * SCH],
                                    start=(ds == 0), stop=(ds == DT - 1))
                    else:
                        for ds in range(DT):
                            nc.tensor.matmul(
                                ps1[:],
                                wsa_t[hi][:, ds * P:(ds + 1) * P],
                                xs_tiles[t][:, ds * SCH:(ds + 1) * SCH],
                                start=(ds == 0), stop=(ds == DT - 1))
                        for ds in range(DT):
                            nc.tensor.matmul(
                                ps2[:],
                                wsa_t[hi][:, DT * P + ds * P:
                                          DT * P + (ds + 1) * P],
                                xs_tiles[t][:, ds * SCH:(ds + 1) * SCH],
                                start=(ds == 0), stop=(ds == DT - 1))
                    t1 = sb.tile([P, 512], F32, tag="t1", bufs=2,
                                 name=f"st1_{t}_{hi}")
                    nc.scalar.activation(t1[:], ps1[:], AF.Silu,
                                         bias=bs_all[:, hi:hi + 1])
                    t2 = sb.tile([P, 512], F32, tag="t2", bufs=2,
                                 name=f"st2_{t}_{hi}")
                    nc.vector.tensor_scalar_add(t2[:], ps2[:],
                                                bs_all[:, HST + hi:HST + hi + 1])
                    st = sb.tile([P, 512], BF16, tag=f"s_{hi}", bufs=2,
                                 name=f"s_{t}_{hi}")
                    nc.vector.tensor_mul(st[:], t1[:], t2[:])
                    s_t[t].append(st)
                    yield

            def gen_shared_sb(t, p_tags, fine=False):
                """(token-tile, D-half) groups; 2 per step, or 1 if fine."""
                groups = [(mm, n2) for mm in range(SCH // P) for n2 in range(DN)]
                step = 1 if fine else 2
                for k in range(0, len(groups), step):
                    for gi, (mm, n2) in enumerate(groups[k:k + step], k):
                        _tg = p_tags[gi // 2 % 2]
                        po = ps.tile([P, 512], F32, tag=_tg,
                                     bufs=4 if _tg == "po" else 2,
                                     name=f"spo_{t}_{mm}_{n2}")
                        for hi in range(HST):
                            nc.tensor.matmul(
                                po[:],
                                s_t[t][hi][:, mm * P:(mm + 1) * P],
                                ws2_t[hi][:, n2 * 512:(n2 + 1) * 512],
                                start=(hi == 0), stop=(hi == HST - 1))
                        z_t = sb.tile([P, 512], BF16, tag="zt", bufs=4,
                                      name=f"z_t{t}_{mm}_{n2}")
                        if gi % 2:
                            nc.vector.tensor_copy(z_t[:], po[:])
                        else:
                            nc.scalar.activation(z_t[:], po[:], AF.Identity)
                        z_eng = nc.sync if fine and gi == 6 else nc.gpsimd
                        z_eng.dma_start(
                            zout[t * (SCH // P) + mm, :, n2 * 512:(n2 + 1) * 512],
                            z_t[:])
                    yield

            def drive(main_gen, fill_gen, sched):
                """Consume steps per explicit schedule string, then drain both."""
                for ch in sched:
                    g = main_gen if ch == "M" else fill_gen
                    try:
                        next(g)
                    except StopIteration:
                        pass
                for g in (main_gen, fill_gen):
                    while True:
                        try:
                            next(g)
                        except StopIteration:
                            break

            def chain(*gens):
                for g in gens:
                    yield from g

            # --- prologue: shared-expert work starts first (it needs only
            # ~145 GB/s of just-in-time DMA vs ~290 for routed), so the PE
            # fills the DMA-constrained bring-up window while the 8MB wa0
            # stream prefetches in the background. Critical pieces go on
            # sync, interleaved in the order SA0 step 0 consumes them. ---
            wsa0 = sb.tile([P, 2 * DT * P], BF16, tag="wsa_0", name="wsa_t0")
            xs0t = sb.tile([P, DT * SCH], BF16, tag="xs", bufs=2,
                           name="xs_t0")
            hx = DT // 2
            nc.sync.dma_start(wsa0[:, :DT * P], wsa[0, :, 0:DT * P])
            nc.sync.dma_start(xs0t[:, :hx * SCH], xs[0:P, 0, 0:hx * SCH])
            nc.sync.dma_start(wsa0[:, DT * P:], wsa[0, :, DT * P:])
            nc.sync.dma_start(xs0t[:, hx * SCH:], xs[0:P, 0, hx * SCH:])
            wsa_t[0] = wsa0
            xs_tiles[0] = xs0t
            # scalar: biases, remaining shared residents
            bs_all = sb.tile([P, 2 * HST], F32, name="bs_all")
            nc.scalar.dma_start(bs_all[:], bsa[:])
            load_wsa(1, nc.scalar)
            b1_all = sb.tile([P, EPC * HT], F32, name="b1_all")
            nc.scalar.dma_start(b1_all[:], b1[:])
            b11_all = sb.tile([P, EPC * HT], F32, name="b11_all")
            nc.scalar.dma_start(b11_all[:], b11[:])
            load_wsa(2, nc.scalar)
            load_wsa(3, nc.scalar)
            g_all = sb.tile([P, EPC * CT], F32, name="g_all")
            nc.scalar.dma_start(g_all[:], gt[:])

            # S1a: SA0+SA1 run alone; fillers prefetch the rA0 stream
            # (wa tiles + xg0 on sync) and later shared inputs.
            def filler_sa0(hi):
                if hi == 1:
                    load_wa(0, 0, nc.sync)
                    load_wa(0, 1, nc.sync)
                elif hi == 2:
                    load_wa(0, 2, nc.sync)
                    load_xs(1, nc.scalar)
                elif hi == 3:
                    load_wa(0, 3, nc.sync)
                    load_xg(0, nc.sync)

            def filler_sa1(hi):
                if hi == 0:
                    load_wa(0, 4, nc.sync)
                    load_wa(0, 5, nc.sync)
                elif hi == 1:
                    load_wa(0, 6, nc.sync)
                    load_wa(0, 7, nc.sync)
                elif hi == 2:
                    load_wa(0, 8, nc.sync)
                    load_wa(0, 9, nc.sync)
                    load_ws2(0, nc.sync)
                    load_ws2(1, nc.sync)
                elif hi == 3:
                    load_wa(0, 10, nc.sync)
                    load_wa(0, 11, nc.sync)
                    load_ws2(2, nc.sync)
                    load_ws2(3, nc.sync)

            for _ in chain(gen_shared_sa(0, ("po", "po"), filler_sa0),
                           gen_shared_sa(1, ("po", "po"), filler_sa1)):
                pass

            # S1b: rA0 runs alone (wa mostly prefetched; tail JIT), with
            # w2(e0) prefetch for S2 and xg1 for S3.
            def filler_a0(hi):
                if hi == 3:
                    load_xg(1, nc.sync)
                elif 8 <= hi < 14:
                    load_w2(0, hi - 8, nc.sync)
            for _ in gen_routed_a(0, filler_a0):
                pass

            # S2: rB0 (streams rest of w2(e0) on scalar) interleaved with
            # SB0+SB1.  sync fillers: prefetch wa(e1) tiles for S3.
            def filler_b0(hi):
                if 2 <= hi < 10:
                    load_wa(1, hi - 2, nc.sync)
                elif hi == 10:
                    load_xs(2, nc.gpsimd)
            sb01 = chain(gen_shared_sb(0, ("p1", "p2")),
                         gen_shared_sb(1, ("p1", "p2")))
            drive(gen_routed_b(0, filler_b0, w2_eng=nc.scalar), sb01,
                  "MMF" * 8 + "M")

            # S3: rA1 (streams wa on sync) interleaved with SA2 then SB2.
            def filler_a1(hi):
                if hi == 4:
                    load_xs(3, nc.gpsimd)
                elif 8 <= hi < 14:
                    load_w2(1, hi - 8, nc.sync)
            sa_sb2 = chain(gen_shared_sa(2, ("po", "po")),
                           gen_shared_sb(2, ("po", "po")))
            drive(gen_routed_a(1, filler_a1), sa_sb2, "MMF" * 8)

            # S4: rB1 (streams rest of w2(e1) on sync) interleaved with
            # SA3 then SB3 (both on p1/p2).
            # SB3 runs at single-group granularity; the last two groups come
            # after the rB1 epilogue so their PE work covers the rout write
            # latency (shorter kernel tail).
            sa_sb3 = chain(gen_shared_sa(3, ("p1", "p2")),
                           gen_shared_sb(3, ("p1", "p2"), fine=True))
            drive(gen_routed_b(1), sa_sb3,
                  "MMF" * 4 + "MF" * 6 + "MMM" + "FF")

    nc.compile()
    return nc


def _route(xf, Wg):
    """Host router: returns (top-k expert ids, gates) per token."""
    logits = xf.astype(np.float64) @ Wg.astype(np.float64)        # [N, E]
    part = np.argpartition(-logits, K - 1, axis=1)[:, :K]          # [N, K]
    pl = np.take_along_axis(logits, part, axis=1)
    order = np.argsort(-pl, axis=1, kind="stable")
    topi = np.take_along_axis(part, order, axis=1)                 # [N, K] sorted
    tl = np.take_along_axis(logits, topi, axis=1)
    m = tl.max(axis=1, keepdims=True)
    e = np.exp(tl - m)
    gates = (e / e.sum(axis=1, keepdims=True)).astype(np.float32)  # [N, K]
    return topi, gates


def kernel(x, Wg, W1, b1, W11, b11, W2, b2, Ws1, bs1, Ws11, bs11, Ws2, bs2,
           _run_opts=None):
    xf = np.ascontiguousarray(x.reshape(N, D), dtype=np.float32)
    topi, gates = _route(xf, Wg)

    # token lists per expert
    flat_e = topi.reshape(-1)                        # [N*K]
    flat_tok = np.repeat(np.arange(N), K)
    flat_g = gates.reshape(-1)
    order = np.argsort(flat_e, kind="stable")
    counts = np.bincount(flat_e, minlength=E)
    starts = np.zeros(E + 1, np.int64)
    np.cumsum(counts, out=starts[1:])
    tok_sorted = flat_tok[order]
    g_sorted = flat_g[order]

    # [P, NCH, DT*SCH]: per chunk t, ds-major contiguous per partition
    xs_arr = np.ascontiguousarray(
        xf.reshape(NCH, SCH, DT, P).transpose(3, 0, 2, 1).reshape(
            P, NCH, DT * SCH)).astype(ml_dtypes.bfloat16)

    in_maps = []
    meta = []          # (expert, idx, g) per (core, j)
    overflow = []      # (expert, idx, g) computed on host
    for c in range(NCORES):
        im = {}
        xg_arr = np.zeros((EPC, P, DT * C), ml_dtypes.bfloat16)
        gt_arr = np.zeros((P, EPC * CT), np.float32)
        wa_arr = np.empty((EPC, HT, P, 2 * DT * P), ml_dtypes.bfloat16)
        w2_arr = np.empty((EPC, HT, P, D), ml_dtypes.bfloat16)
        b1_arr = np.empty((P, EPC * HT), np.float32)
        b11_arr = np.empty((P, EPC * HT), np.float32)
        core_meta = []
        for j in range(EPC):
            e_id = c * EPC + j
            idx = tok_sorted[starts[e_id]:starts[e_id + 1]]
            g = g_sorted[starts[e_id]:starts[e_id + 1]]
            if len(idx) > C:
                overflow.append((e_id, idx[C:], g[C:]))
                idx, g = idx[:C], g[:C]
            n_e = len(idx)
            core_meta.append((e_id, idx, g))
            # gathered tokens, transposed: [P, DT, C]
            xpad = np.zeros((C, D), np.float32)
            xpad[:n_e] = xf[idx]
            xg_arr[j] = xpad.reshape(C, DT, P).transpose(2, 1, 0).reshape(
                P, DT * C).astype(ml_dtypes.bfloat16)
            gpad = np.zeros(CT * P, np.float32)
            gpad[:n_e] = g
            gt_arr[:, j * CT:(j + 1) * CT] = gpad.reshape(CT, P).T
            wa_arr[j, :, :, :DT * P] = np.asarray(W1[e_id]).reshape(
                DT, P, HT, P).transpose(2, 1, 0, 3).reshape(
                HT, P, DT * P).astype(ml_dtypes.bfloat16)
            wa_arr[j, :, :, DT * P:] = np.asarray(W11[e_id]).reshape(
                DT, P, HT, P).transpose(2, 1, 0, 3).reshape(
                HT, P, DT * P).astype(ml_dtypes.bfloat16)
            w2_arr[j] = np.asarray(W2[e_id]).reshape(
                HT, P, D).astype(ml_dtypes.bfloat16)
            b1_arr[:, j * HT:(j + 1) * HT] = \
                np.asarray(b1[e_id], np.float32).reshape(HT, P).T
            b11_arr[:, j * HT:(j + 1) * HT] = \
                np.asarray(b11[e_id], np.float32).reshape(HT, P).T
        meta.append(core_meta)
        im["xg"] = xg_arr
        im["gt"] = gt_arr
        im["wa"] = wa_arr
        im["w2"] = w2_arr
        im["b1"] = b1_arr
        im["b11"] = b11_arr
        # shared expert slice
        sl = slice(c * HSS, (c + 1) * HSS)
        im["xs"] = xs_arr
        wsa_arr = np.empty((HST, P, 2 * DT * P), ml_dtypes.bfloat16)
        wsa_arr[:, :, :DT * P] = np.asarray(Ws1)[:, sl].reshape(
            DT, P, HST, P).transpose(2, 1, 0, 3).reshape(
            HST, P, DT * P).astype(ml_dtypes.bfloat16)
        wsa_arr[:, :, DT * P:] = np.asarray(Ws11)[:, sl].reshape(
            DT, P, HST, P).transpose(2, 1, 0, 3).reshape(
            HST, P, DT * P).astype(ml_dtypes.bfloat16)
        im["wsa"] = wsa_arr
        im["ws2"] = np.asarray(Ws2)[sl].reshape(
            HST, P, D).astype(ml_dtypes.bfloat16)
        bsa_arr = np.empty((P, 2 * HST), np.float32)
        bsa_arr[:, :HST] = np.asarray(bs1, np.float32)[sl].reshape(HST, P).T
        bsa_arr[:, HST:] = np.asarray(bs11, np.float32)[sl].reshape(HST, P).T
        im["bsa"] = bsa_arr
        in_maps.append(im)

    if "nc" not in _CACHED:
        _CACHED["nc"] = _build_nc()
    nc = _CACHED["nc"]

    run_opts = _run_opts or {}
    res = bass_utils.run_bass_kernel_spmd(
        nc, in_maps, core_ids=list(range(NCORES)), **run_opts)
    _CACHED["last_results"] = res

    # ---- host-side unshard / combine ----
    y = np.zeros((N, D), np.float32)
    for c in range(NCORES):
        ro = np.asarray(res.results[c]["rout"], np.float32).reshape(
            EPC, CT * P, D)
        for j in range(EPC):
            e_id, idx, g = meta[c][j]
            n_e = len(idx)
            np.add.at(y, idx, ro[j, :n_e] + g[:, None] * b2[e_id][None, :])
        zc = np.asarray(res.results[c]["zout"], np.float32).reshape(N, D)
        if c == 0:
            z = zc
        else:
            z += zc

    for e_id, idx, g in overflow:
        xo = xf[idx]
        h = _silu(xo @ W1[e_id] + b1[e_id]) * (xo @ W11[e_id] + b11[e_id])
        np.add.at(y, idx, (h @ W2[e_id] + b2[e_id]) * g[:, None])

    out = y + z + np.asarray(bs2, np.float32)[None, :]
    return out.reshape(B, T, D).astype(np.float32)


def _silu(v):
    return v * (1.0 / (1.0 + np.exp(-v)))


# revision 58
# speedup vs baseline: 1.0428x; 1.0369x over previous
"""Trainium2 Bass kernel for a 16-expert top-2 MoE layer with shared expert.

Sharding (8 cores):
  - Routed experts: expert-parallel, 2 experts per core, host-side token
    dispatch (gather) with a static per-expert capacity of C tokens.
  - Shared expert: tensor-parallel over the 4096-wide intermediate dim
    (512 per core); partial outputs summed on host.
  - Router runs on host (tiny: [2048,1024]@[1024,16]).

All matmul operands run in bf16 (fp32 PSUM accumulation); end-to-end
relative error ~4e-3.  bf16 halves every weight stream vs the fp32r
baseline and enables fast-weight-load on all stationaries, which takes
DMA off the critical path: the kernel is PE-issue-bound at ~1 moving
column/cycle.

Program order interleaves shared-expert chunks between routed-expert
steps so the PE never idles while weights stream in, and weight streams
for the next phase are prefetched during the previous one (wA1 during
S2, w2(e0) during late S1) to avoid phase-boundary stalls.
Streamed routed weights go on the sync HWDGE queue; activations,
resident shared weights and the e0 w2 stream on the scalar HWDGE queue;
outputs on the gpsimd SWDGE queue.
"""
import sys
sys.path.insert(0, "/opt/trn_rl_repo")
import numpy as np
import ml_dtypes
from concourse import bacc, mybir
from concourse import tile
from concourse import bass_utils

# Problem shape (hardcoded per contract)
B, T, D = 2, 1024, 1024
N = B * T                # 2048 tokens
E = 16                   # routed experts
H = 2048                 # expert intermediate
K = 2                    # top-k
HS = 4096                # shared intermediate
NCORES = 8
EPC = E // NCORES        # experts per core = 2
HSS = HS // NCORES       # shared intermediate slice per core = 512

P = 128
C = 256                  # per-expert token capacity (mean load is 256)
CT = (C + P - 1) // P    # 2 token tiles per expert
DT = D // P              # 8 contraction slices over D
HT = H // P              # 16 H tiles
HST = HSS // P           # 4 shared-H tiles per core
SCH = 512                # shared-expert token chunk (moving dim)
NCH = N // SCH           # 4 chunks
DN = D // 512            # 2 output free tiles of 512

F32 = mybir.dt.float32
BF16 = mybir.dt.bfloat16
AF = mybir.ActivationFunctionType

_CACHED = {}


def _mtiles():
    """Token tiles of (offset, width) covering C in 128-row steps."""
    out = []
    off = 0
    while off < C:
        out.append((off, min(P, C - off)))
        off += P
    return out


def _build_nc():
    nc = bacc.Bacc("TRN2", target_bir_lowering=False, debug=False,
                   num_devices=NCORES)

    # ---- DRAM I/O (per-core) ----
    xg = nc.dram_tensor("xg", [EPC, P, DT * C], BF16, kind="ExternalInput").ap()
    gt = nc.dram_tensor("gt", [P, EPC * CT], F32, kind="ExternalInput").ap()
    wa = nc.dram_tensor("wa", [EPC, HT, P, 2 * DT * P], BF16,
                        kind="ExternalInput").ap()
    w2 = nc.dram_tensor("w2", [EPC, HT, P, D], BF16, kind="ExternalInput").ap()
    b1 = nc.dram_tensor("b1", [P, EPC * HT], F32, kind="ExternalInput").ap()
    b11 = nc.dram_tensor("b11", [P, EPC * HT], F32, kind="ExternalInput").ap()
    xs = nc.dram_tensor("xs", [P, NCH, DT * SCH], BF16,
                        kind="ExternalInput").ap()
    wsa = nc.dram_tensor("wsa", [HST, P, 2 * DT * P], BF16,
                         kind="ExternalInput").ap()
    ws2 = nc.dram_tensor("ws2", [HST, P, D], BF16, kind="ExternalInput").ap()
    bsa = nc.dram_tensor("bsa", [P, 2 * HST], F32, kind="ExternalInput").ap()
    rout = nc.dram_tensor("rout", [EPC, CT, P, D], BF16,
                          kind="ExternalOutput").ap()
    zout = nc.dram_tensor("zout", [N // P, P, D], BF16,
                          kind="ExternalOutput").ap()

    mtiles = _mtiles()

    with tile.TileContext(nc) as tc:
        with tc.tile_pool(name="sb", bufs=1) as sb, \
             tc.tile_pool(name="ps", bufs=1, space="PSUM") as ps:

            xg_t = [None, None]

            def load_xg(j, eng, split=False):
                t = sb.tile([P, DT * C], BF16, tag=f"xg{j}", name=f"xg_t{j}")
                if split:
                    eng.dma_start(t[:, :DT * C // 2], xg[j, :, 0:DT * C // 2])
                    eng.dma_start(t[:, DT * C // 2:], xg[j, :, DT * C // 2:])
                else:
                    eng.dma_start(t[:], xg[j])
                xg_t[j] = t

            wa_t = {}

            def load_wa(j, hi, eng, split=False):
                t = sb.tile([P, 2 * DT * P], BF16, tag="wa", bufs=12,
                            name=f"wa_t{j}_{hi}")
                if split:
                    eng.dma_start(t[:, :DT * P], wa[j, hi, :, 0:DT * P])
                    eng.dma_start(t[:, DT * P:], wa[j, hi, :, DT * P:])
                else:
                    eng.dma_start(t[:], wa[j, hi])
                wa_t[(j, hi)] = t

            w2_t = {}

            def load_w2(j, hi, eng):
                t = sb.tile([P, D], BF16, tag="w2", bufs=8,
                            name=f"w2_t{j}_{hi}")
                eng.dma_start(t[:], w2[j, hi])
                w2_t[(j, hi)] = t

            wsa_t = [None] * HST
            ws2_t = [None] * HST

            def load_wsa(hi, eng):
                t = sb.tile([P, 2 * DT * P], BF16, tag=f"wsa_{hi}",
                            name=f"wsa_t{hi}")
                eng.dma_start(t[:], wsa[hi])
                wsa_t[hi] = t

            def load_ws2(hi, eng):
                t = sb.tile([P, D], BF16, tag=f"ws2_{hi}", name=f"ws2_t{hi}")
                eng.dma_start(t[:], ws2[hi])
                ws2_t[hi] = t

            xs_tiles = {}

            def load_xs(t, eng, split=1):
                xs_t = sb.tile([P, DT * SCH], BF16, tag="xs", bufs=2,
                               name=f"xs_t{t}")
                qw = DT * SCH // split
                for q in range(split):
                    eng.dma_start(xs_t[:, q * qw:(q + 1) * qw],
                                  xs[:, t, q * qw:(q + 1) * qw])
                xs_tiles[t] = xs_t

            h_t = {0: [], 1: []}
            s_t = {}

            def gen_routed_a(j, dma_filler=None):
                """16 steps, one H-tile each: stream wa, 2 psum groups,
                silu/bias-add/mul -> hT[hi] (bf16)."""
                for hi in range(HT):
                    if (j, hi) not in wa_t:
                        load_wa(j, hi, nc.sync)
                    if dma_filler is not None:
                        dma_filler(hi)
                    wt = wa_t[(j, hi)]
                    ps1 = ps.tile([P, 512], F32, tag="p1", bufs=2,
                                  name=f"ps1_{j}_{hi}")
                    ps2 = ps.tile([P, 512], F32, tag="p2", bufs=2,
                                  name=f"ps2_{j}_{hi}")
                    if j == 0 and hi == 0:
                        # first step: emit in DMA piece-arrival order
                        # (w1 half, xg half 1, w11 half, xg half 2)
                        halves = [(ps1, 0, range(0, DT // 2)),
                                  (ps2, DT * P, range(0, DT // 2)),
                                  (ps1, 0, range(DT // 2, DT)),
                                  (ps2, DT * P, range(DT // 2, DT))]
                        for pst, woff, dss in halves:
                            for ds in dss:
                                nc.tensor.matmul(
                                    pst[:, :C],
                                    wt[:, woff + ds * P:woff + (ds + 1) * P],
                                    xg_t[j][:, ds * C:(ds + 1) * C],
                                    start=(ds == 0), stop=(ds == DT - 1))
                    else:
                        for ds in range(DT):
                            nc.tensor.matmul(
                                ps1[:, :C],
                                wt[:, ds * P:(ds + 1) * P],
                                xg_t[j][:, ds * C:(ds + 1) * C],
                                start=(ds == 0), stop=(ds == DT - 1))
                        for ds in range(DT):
                            nc.tensor.matmul(
                                ps2[:, :C],
                                wt[:, DT * P + ds * P:DT * P + (ds + 1) * P],
                                xg_t[j][:, ds * C:(ds + 1) * C],
                                start=(ds == 0), stop=(ds == DT - 1))
                    t1 = sb.tile([P, C], F32, tag="t1r", bufs=2,
                                 name=f"t1_{j}_{hi}")
                    nc.scalar.activation(t1[:], ps1[:, :C], AF.Silu,
                                         bias=b1_all[:, j * HT + hi:j * HT + hi + 1])
                    t2 = sb.tile([P, C], F32, tag="t2r", bufs=2,
                                 name=f"t2_{j}_{hi}")
                    nc.vector.tensor_scalar_add(
                        t2[:], ps2[:, :C],
                        b11_all[:, j * HT + hi:j * HT + hi + 1])
                    ht = sb.tile([P, C], BF16, tag=f"h_{hi}", name=f"h_{j}_{hi}")
                    nc.vector.tensor_mul(ht[:], t1[:], t2[:])
                    h_t[j].append(ht)
                    yield

            def gen_routed_b(j, dma_filler=None, w2_eng=None):
                """16 steps (one per streamed w2 tile) + epilogue step."""
                pos = [[ps.tile([P, 512], F32, tag="po", bufs=4,
                                name=f"po_{j}_{m}_{n2}")
                        for n2 in range(DN)] for m in range(len(mtiles))]
                for hi in range(HT):
                    if (j, hi) not in w2_t:
                        load_w2(j, hi, w2_eng or nc.sync)
                    if dma_filler is not None:
                        dma_filler(hi)
                    w2t = w2_t[(j, hi)]
                    for m, (off, mw) in enumerate(mtiles):
                        for n2 in range(DN):
                            nc.tensor.matmul(
                                pos[m][n2][:mw, :],
                                h_t[j][hi][:, off:off + mw],
                                w2t[:, n2 * 512:(n2 + 1) * 512],
                                start=(hi == 0), stop=(hi == HT - 1))
                    yield
                spread = j == EPC - 1   # final expert: minimize kernel tail
                for i, (m, n2) in enumerate(
                        (m, n2) for m in range(len(mtiles)) for n2 in range(DN)):
                    off, mw = mtiles[m]
                    o_t = sb.tile([P, 512], BF16, tag="ot", bufs=4,
                                  name=f"o_t{j}_{m}_{n2}")
                    gsc = g_all[:mw, j * CT + m:j * CT + m + 1]
                    if spread and i >= 2:
                        nc.vector.tensor_scalar_mul(
                            o_t[:mw, :], pos[m][n2][:mw, :], gsc)
                    else:
                        nc.scalar.activation(
                            o_t[:mw, :], pos[m][n2][:mw, :], AF.Identity,
                            scale=gsc)
                    oe = nc.gpsimd
                    if spread:
                        oe = nc.sync if i < 2 else nc.gpsimd
                    oe.dma_start(
                        rout[j, m, 0:mw, n2 * 512:(n2 + 1) * 512],
                        o_t[:mw, :])
                yield

            def gen_shared_sa(t, p_tags, dma_filler=None):
                """4 steps, one shared-H tile each."""
                s_t[t] = []
                for hi in range(HST):
                    if dma_filler is not None:
                        dma_filler(hi)
                    ps1 = ps.tile([P, 512], F32, tag=p_tags[0],
                                  bufs=4 if p_tags[0] == "po" else 2,
                                  name=f"sps1_{t}_{hi}")
                    ps2 = ps.tile([P, 512], F32, tag=p_tags[1],
                                  bufs=4 if p_tags[1] == "po" else 2,
                                  name=f"sps2_{t}_{hi}")
                    if t == 0 and hi == 0:
                        # first step: emit in DMA piece-arrival order
                        # (wsa h1, xs h1, wsa h2, xs h2)
                        halves = [(ps1, 0, range(0, DT // 2)),
                                  (ps2, DT * P, range(0, DT // 2)),
                                  (ps1, 0, range(DT // 2, DT)),
                                  (ps2, DT * P, range(DT // 2, DT))]
                        for pst, woff, dss in halves:
                            for ds in dss:
                                nc.tensor.matmul(
                                    pst[:],
                                    wsa_t[hi][:, woff + ds * P:
                                              woff + (ds + 1) * P],
                                    xs_tiles[t][:, ds * SCH:(ds + 1) * SCH],
                                    start=(ds == 0), stop=(ds == DT - 1))
                    else:
                        for ds in range(DT):
                            nc.tensor.matmul(
                                ps1[:],
                                wsa_t[hi][:, ds * P:(ds + 1) * P],
                                xs_tiles[t][:, ds * SCH:(ds + 1) * SCH],
                                start=(ds == 0), stop=(ds == DT - 1))
                        for ds in range(DT):
                            nc.tensor.matmul(
                                ps2[:],
                                wsa_t[hi][:, DT * P + ds * P:
                                          DT * P + (ds + 1) * P],
                                xs_tiles[t][:, ds * SCH:(ds + 1) * SCH],
                                start=(ds == 0), stop=(ds == DT - 1))
                    t1 = sb.tile([P, 512], F32, tag="t1", bufs=2,
                                 name=f"st1_{t}_{hi}")
                    nc.scalar.activation(t1[:], ps1[:], AF.Silu,
                                         bias=bs_all[:, hi:hi + 1])
                    t2 = sb.tile([P, 512], F32, tag="t2", bufs=2,
                                 name=f"st2_{t}_{hi}")
                    nc.vector.tensor_scalar_add(t2[:], ps2[:],
                                                bs_all[:, HST + hi:HST + hi + 1])
                    st = sb.tile([P, 512], BF16, tag=f"s_{hi}", bufs=2,
                                 name=f"s_{t}_{hi}")
                    nc.vector.tensor_mul(st[:], t1[:], t2[:])
                    s_t[t].append(st)
                    yield

            def gen_shared_sb(t, p_tags, fine=False):
                """(token-tile, D-half) groups; 2 per step, or 1 if fine."""
                groups = [(mm, n2) for mm in range(SCH // P) for n2 in range(DN)]
                step = 1 if fine else 2
                for k in range(0, len(groups), step):
                    for gi, (mm, n2) in enumerate(groups[k:k + step], k):
                        _tg = p_tags[gi // 2 % 2]
                        po = ps.tile([P, 512], F32, tag=_tg,
                                     bufs=4 if _tg == "po" else 2,
                                     name=f"spo_{t}_{mm}_{n2}")
                        for hi in range(HST):
                            nc.tensor.matmul(
                                po[:],
                                s_t[t][hi][:, mm * P:(mm + 1) * P],
                                ws2_t[hi][:, n2 * 512:(n2 + 1) * 512],
                                start=(hi == 0), stop=(hi == HST - 1))
                        z_t = sb.tile([P, 512], BF16, tag="zt", bufs=4,
                                      name=f"z_t{t}_{mm}_{n2}")
                        if gi % 2:
                            nc.vector.tensor_copy(z_t[:], po[:])
                        else:
                            nc.scalar.activation(z_t[:], po[:], AF.Identity)
                        z_eng = nc.sync if fine and gi == 6 else nc.gpsimd
                        z_eng.dma_start(
                            zout[t * (SCH // P) + mm, :, n2 * 512:(n2 + 1) * 512],
                            z_t[:])
                    yield

            def drive(main_gen, fill_gen, sched):
                """Consume steps per explicit schedule string, then drain both."""
                for ch in sched:
                    g = main_gen if ch == "M" else fill_gen
                    try:
                        next(g)
                    except StopIteration:
                        pass
                for g in (main_gen, fill_gen):
                    while True:
                        try:
                            next(g)
                        except StopIteration:
                            break

            def chain(*gens):
                for g in gens:
                    yield from g

            # --- prologue: shared-expert work starts first (it needs only
            # ~145 GB/s of just-in-time DMA vs ~290 for routed), so the PE
            # fills the DMA-constrained bring-up window while the 8MB wa0
            # stream prefetches in the background. Critical pieces go on
            # sync, interleaved in the order SA0 step 0 consumes them. ---
            wsa0 = sb.tile([P, 2 * DT * P], BF16, tag="wsa_0", name="wsa_t0")
            xs0t = sb.tile([P, DT * SCH], BF16, tag="xs", bufs=2,
                           name="xs_t0")
            hx = DT // 2
            nc.sync.dma_start(wsa0[:, :DT * P], wsa[0, :, 0:DT * P])
            nc.sync.dma_start(xs0t[:, :hx * SCH], xs[0:P, 0, 0:hx * SCH])
            nc.sync.dma_start(wsa0[:, DT * P:], wsa[0, :, DT * P:])
            nc.sync.dma_start(xs0t[:, hx * SCH:], xs[0:P, 0, hx * SCH:])
            wsa_t[0] = wsa0
            xs_tiles[0] = xs0t
            # sync (continued): biases, remaining shared residents.  The
            # scalar engine issues NO DMAs anywhere in this kernel: a
            # dma_start can block on the DMA in-flight-depth gate, and at
            # the head of the scalar FIFO that starves the ACTs the PE
            # needs for PSUM recycling.
            bs_all = sb.tile([P, 2 * HST], F32, name="bs_all")
            nc.sync.dma_start(bs_all[:], bsa[:])
            load_wsa(1, nc.sync)
            b1_all = sb.tile([P, EPC * HT], F32, name="b1_all")
            nc.sync.dma_start(b1_all[:], b1[:])
            b11_all = sb.tile([P, EPC * HT], F32, name="b11_all")
            nc.sync.dma_start(b11_all[:], b11[:])
            load_wsa(2, nc.sync)
            load_wsa(3, nc.sync)
            g_all = sb.tile([P, EPC * CT], F32, name="g_all")
            nc.sync.dma_start(g_all[:], gt[:])

            # S1a: SA0+SA1 run alone; fillers prefetch the rA0 stream
            # (wa tiles + xg0 on sync) and later shared inputs.
            def filler_sa0(hi):
                if hi == 1:
                    load_wa(0, 0, nc.sync)
                    load_wa(0, 1, nc.sync)
                elif hi == 2:
                    load_wa(0, 2, nc.sync)
                    load_xs(1, nc.sync)
                elif hi == 3:
                    load_wa(0, 3, nc.sync)
                    load_xg(0, nc.sync)

            def filler_sa1(hi):
                if hi == 0:
                    load_wa(0, 4, nc.sync)
                    load_wa(0, 5, nc.sync)
                elif hi == 1:
                    load_wa(0, 6, nc.sync)
                    load_wa(0, 7, nc.sync)
                elif hi == 2:
                    load_wa(0, 8, nc.sync)
                    load_wa(0, 9, nc.sync)
                    load_ws2(0, nc.sync)
                    load_ws2(1, nc.sync)
                elif hi == 3:
                    load_wa(0, 10, nc.sync)
                    load_wa(0, 11, nc.sync)
                    load_ws2(2, nc.sync)
                    load_ws2(3, nc.sync)

            for _ in chain(gen_shared_sa(0, ("po", "po"), filler_sa0),
                           gen_shared_sa(1, ("po", "po"), filler_sa1)):
                pass

            # S1b: rA0 runs alone (wa mostly prefetched; tail JIT), with
            # w2(e0) prefetch for S2 and xg1 for S3.
            def filler_a0(hi):
                if hi == 3:
                    load_xg(1, nc.sync)
                elif 8 <= hi < 14:
                    load_w2(0, hi - 8, nc.sync)
            for _ in gen_routed_a(0, filler_a0):
                pass

            # S2: rB0 (streams rest of w2(e0) on scalar) interleaved with
            # SB0+SB1.  sync fillers: prefetch wa(e1) tiles for S3.
            def filler_b0(hi):
                if 2 <= hi < 10:
                    load_wa(1, hi - 2, nc.sync)
                elif hi == 10:
                    load_xs(2, nc.gpsimd)
            sb01 = chain(gen_shared_sb(0, ("p1", "p2")),
                         gen_shared_sb(1, ("p1", "p2")))
            drive(gen_routed_b(0, filler_b0), sb01, "MMF" * 8 + "M")

            # S3: rA1 (streams wa on sync) interleaved with SA2 then SB2.
            def filler_a1(hi):
                if hi == 4:
                    load_xs(3, nc.gpsimd)
                elif 8 <= hi < 14:
                    load_w2(1, hi - 8, nc.sync)
            sa_sb2 = chain(gen_shared_sa(2, ("po", "po")),
                           gen_shared_sb(2, ("po", "po")))
            drive(gen_routed_a(1, filler_a1), sa_sb2, "MMF" * 8)

            # S4: rB1 (streams rest of w2(e1) on sync) interleaved with
            # SA3 then SB3 (both on p1/p2).
            # SB3 runs at single-group granularity; the last two groups come
            # after the rB1 epilogue so their PE work covers the rout write
            # latency (shorter kernel tail).
            sa_sb3 = chain(gen_shared_sa(3, ("p1", "p2")),
                           gen_shared_sb(3, ("p1", "p2"), fine=True))
            drive(gen_routed_b(1), sa_sb3,
                  "MMF" * 4 + "MF" * 6 + "MMM" + "FF")

    nc.compile()
    return nc


def _route(xf, Wg):
    """Host router: returns (top-k expert ids, gates) per token."""
    logits = xf.astype(np.float64) @ Wg.astype(np.float64)        # [N, E]
    part = np.argpartition(-logits, K - 1, axis=1)[:, :K]          # [N, K]
    pl = np.take_along_axis(logits, part, axis=1)
    order = np.argsort(-pl, axis=1, kind="stable")
    topi = np.take_along_axis(part, order, axis=1)                 # [N, K] sorted
    tl = np.take_along_axis(logits, topi, axis=1)
    m = tl.max(axis=1, keepdims=True)
    e = np.exp(tl - m)
    gates = (e / e.sum(axis=1, keepdims=True)).astype(np.float32)  # [N, K]
    return topi, gates


def kernel(x, Wg, W1, b1, W11, b11, W2, b2, Ws1, bs1, Ws11, bs11, Ws2, bs2,
           _run_opts=None):
    xf = np.ascontiguousarray(x.reshape(N, D), dtype=np.float32)
    topi, gates = _route(xf, Wg)

    # token lists per expert
    flat_e = topi.reshape(-1)                        # [N*K]
    flat_tok = np.repeat(np.arange(N), K)
    flat_g = gates.reshape(-1)
    order = np.argsort(flat_e, kind="stable")
    counts = np.bincount(flat_e, minlength=E)
    starts = np.zeros(E + 1, np.int64)
    np.cumsum(counts, out=starts[1:])
    tok_sorted = flat_tok[order]
    g_sorted = flat_g[order]

    # [P, NCH, DT*SCH]: per chunk t, ds-major contiguous per partition
    xs_arr = np.ascontiguousarray(
        xf.reshape(NCH, SCH, DT, P).transpose(3, 0, 2, 1).reshape(
            P, NCH, DT * SCH)).astype(ml_dtypes.bfloat16)

    in_maps = []
    meta = []          # (expert, idx, g) per (core, j)
    overflow = []      # (expert, idx, g) computed on host
    for c in range(NCORES):
        im = {}
        xg_arr = np.zeros((EPC, P, DT * C), ml_dtypes.bfloat16)
        gt_arr = np.zeros((P, EPC * CT), np.float32)
        wa_arr = np.empty((EPC, HT, P, 2 * DT * P), ml_dtypes.bfloat16)
        w2_arr = np.empty((EPC, HT, P, D), ml_dtypes.bfloat16)
        b1_arr = np.empty((P, EPC * HT), np.float32)
        b11_arr = np.empty((P, EPC * HT), np.float32)
        core_meta = []
        for j in range(EPC):
            e_id = c * EPC + j
            idx = tok_sorted[starts[e_id]:starts[e_id + 1]]
            g = g_sorted[starts[e_id]:starts[e_id + 1]]
            if len(idx) > C:
                overflow.append((e_id, idx[C:], g[C:]))
                idx, g = idx[:C], g[:C]
            n_e = len(idx)
            core_meta.append((e_id, idx, g))
            # gathered tokens, transposed: [P, DT, C]
            xpad = np.zeros((C, D), np.float32)
            xpad[:n_e] = xf[idx]
            xg_arr[j] = xpad.reshape(C, DT, P).transpose(2, 1, 0).reshape(
                P, DT * C).astype(ml_dtypes.bfloat16)
            gpad = np.zeros(CT * P, np.float32)
            gpad[:n_e] = g
            gt_arr[:, j * CT:(j + 1) * CT] = gpad.reshape(CT, P).T
            wa_arr[j, :, :, :DT * P] = np.asarray(W1[e_id]).reshape(
                DT, P, HT, P).transpose(2, 1, 0, 3).reshape(
                HT, P, DT * P).astype(ml_dtypes.bfloat16)
            wa_arr[j, :, :, DT * P:] = np.asarray(W11[e_id]).reshape(
                DT, P, HT, P).transpose(2, 1, 0, 3).reshape(
                HT, P, DT * P).astype(ml_dtypes.bfloat16)
            w2_arr[j] = np.asarray(W2[e_id]).reshape(
                HT, P, D).astype(ml_dtypes.bfloat16)
            b1_arr[:, j * HT:(j + 1) * HT] = \
                np.asarray(b1[e_id], np.float32).reshape(HT, P).T
            b11_arr[:, j * HT:(j + 1) * HT] = \
                np.asarray(b11[e_id], np.float32).reshape(HT, P).T
        meta.append(core_meta)
        im["xg"] = xg_arr
        im["gt"] = gt_arr
        im["wa"] = wa_arr
        im["w2"] = w2_arr
        im["b1"] = b1_arr
        im["b11"] = b11_arr
        # shared expert slice
        sl = slice(c * HSS, (c + 1) * HSS)
        im["xs"] = xs_arr
        wsa_arr = np.empty((HST, P, 2 * DT * P), ml_dtypes.bfloat16)
        wsa_arr[:, :, :DT * P] = np.asarray(Ws1)[:, sl].reshape(
            DT, P, HST, P).transpose(2, 1, 0, 3).reshape(
            HST, P, DT * P).astype(ml_dtypes.bfloat16)
        wsa_arr[:, :, DT * P:] = np.asarray(Ws11)[:, sl].reshape(
            DT, P, HST, P).transpose(2, 1, 0, 3).reshape(
            HST, P, DT * P).astype(ml_dtypes.bfloat16)
        im["wsa"] = wsa_arr
        im["ws2"] = np.asarray(Ws2)[sl].reshape(
            HST, P, D).astype(ml_dtypes.bfloat16)
        bsa_arr = np.empty((P, 2 * HST), np.float32)
        bsa_arr[:, :HST] = np.asarray(bs1, np.float32)[sl].reshape(HST, P).T
        bsa_arr[:, HST:] = np.asarray(bs11, np.float32)[sl].reshape(HST, P).T
        im["bsa"] = bsa_arr
        in_maps.append(im)

    if "nc" not in _CACHED:
        _CACHED["nc"] = _build_nc()
    nc = _CACHED["nc"]

    run_opts = _run_opts or {}
    res = bass_utils.run_bass_kernel_spmd(
        nc, in_maps, core_ids=list(range(NCORES)), **run_opts)
    _CACHED["last_results"] = res

    # ---- host-side unshard / combine ----
    y = np.zeros((N, D), np.float32)
    for c in range(NCORES):
        ro = np.asarray(res.results[c]["rout"], np.float32).reshape(
            EPC, CT * P, D)
        for j in range(EPC):
            e_id, idx, g = meta[c][j]
            n_e = len(idx)
            np.add.at(y, idx, ro[j, :n_e] + g[:, None] * b2[e_id][None, :])
        zc = np.asarray(res.results[c]["zout"], np.float32).reshape(N, D)
        if c == 0:
            z = zc
        else:
            z += zc

    for e_id, idx, g in overflow:
        xo = xf[idx]
        h = _silu(xo @ W1[e_id] + b1[e_id]) * (xo @ W11[e_id] + b11[e_id])
        np.add.at(y, idx, (h @ W2[e_id] + b2[e_id]) * g[:, None])

    out = y + z + np.asarray(bs2, np.float32)[None, :]
    return out.reshape(B, T, D).astype(np.float32)


def _silu(v):
    return v * (1.0 / (1.0 + np.exp(-v)))
